# revision 13
# baseline (speedup 1.0000x reference)
"""Trainium2 Bass kernel for the DEAM dense-CNN block.

Data-parallel over batch: 16 samples -> 8 cores x 2 samples.
Per sample: attention chain (GAP -> conv1d -> sigmoid/softmax heads),
dynamic per-sample 3x3 conv as 9 shifted matmuls (fp32r), LGA gating
branch, fused add + batch BN (cross-core AllReduce of per-channel
sums) + ReLU.

Channel shuffle is folded into host-side weight permutations plus a
permuted output DMA, so no on-chip data movement is spent on it.
"""

import numpy as np

import concourse.bass as bass
import concourse.mybir as mybir
import concourse.tile as tile
from concourse import bacc
from concourse.bass_utils import run_bass_kernel_spmd
from concourse.masks import make_identity

F32 = mybir.dt.float32
F32R = mybir.dt.float32r
AX = mybir.AxisListType
ALU = mybir.AluOpType
ACT = mybir.ActivationFunctionType

B, C, H, W = 16, 256, 64, 64
HW = H * W
KNUM, KS = 4, 3
N_CORES = 8
B_LOC = B // N_CORES          # 2 samples per core
NT = C // 128                 # 2 channel tiles
BN_EPS = 1e-5
XW = W + 2                    # padded row width 66
PQ = KS * KS                  # 9
EFREE = PQ * C                # 2304 free elems of an agg/E tile
EHALF = EFREE // 2            # 1152

# shift order: (0,0) first so the start=True matmul covers the full bank
SHIFTS = [(0, 0), (0, -1), (0, 1), (-1, -1), (-1, 0), (-1, 1),
          (1, -1), (1, 0), (1, 1)]


def build_program():
    nc = bacc.Bacc("TRN2", target_bir_lowering=False, debug=False,
                   num_devices=N_CORES)

    x_d = nc.dram_tensor("x", [B_LOC, C, H, XW], F32R,
                        kind="ExternalInput")
    e_d = nc.dram_tensor("ew", [KNUM, NT, 128, EFREE], F32,
                         kind="ExternalInput")
    aow_d = nc.dram_tensor("aow", [NT, 128, C], F32, kind="ExternalInput")
    akw_d = nc.dram_tensor("akw", [NT, 128, KNUM], F32, kind="ExternalInput")
    w1t_d = nc.dram_tensor("w1t", [128, 16], F32, kind="ExternalInput")
    w2t_d = nc.dram_tensor("w2t", [16, 128], F32, kind="ExternalInput")
    gb_d = nc.dram_tensor("gb", [NT, 128, 2], F32, kind="ExternalInput")
    sm_d = nc.dram_tensor("sm", [1, 8], F32, kind="ExternalInput")
    out_d = nc.dram_tensor("out", [B_LOC, C, H, W], F32,
                           kind="ExternalOutput")

    with tile.TileContext(nc) as tc:
        with (
            tc.tile_pool(name="singles", bufs=1) as singles,
            tc.tile_pool(name="xq", bufs=2 * B_LOC) as xq_pool,
            tc.tile_pool(name="z", bufs=2 * B_LOC) as z_pool,
            tc.tile_pool(name="ep", bufs=2) as e_pool,
            tc.tile_pool(name="agg", bufs=2 * B_LOC) as agg_pool,
            tc.tile_pool(name="sp", bufs=2) as sp,
            tc.tile_pool(name="psc", bufs=4, space="PSUM") as ps_conv,
            tc.tile_pool(name="pss", bufs=2, space="PSUM") as pss,
            tc.tile_pool(name="dram", bufs=2, space="DRAM") as dram,
        ):
            # ---- constants ----
            ident = singles.tile([128, 128], F32, tag="ident")
            make_identity(nc, ident[:, :])
            smalls = singles.tile([1, 8], F32, tag="smalls")
            nc.sync.dma_start(out=smalls[:, :], in_=sm_d[:, :])
            aow_s = []
            akw_s = []
            for t in range(NT):
                a = singles.tile([128, C], F32, tag=f"aow{t}")
                nc.sync.dma_start(out=a[:, :], in_=aow_d[t])
                aow_s.append(a)
                k = singles.tile([128, KNUM], F32, tag=f"akw{t}")
                nc.sync.dma_start(out=k[:, :], in_=akw_d[t])
                akw_s.append(k)
            w1t_s = singles.tile([128, 16], F32, tag="w1t")
            nc.sync.dma_start(out=w1t_s[:, :], in_=w1t_d[:, :])
            w2t_s = singles.tile([16, 128], F32, tag="w2t")
            nc.sync.dma_start(out=w2t_s[:, :], in_=w2t_d[:, :])
            gb_s = singles.tile([128, NT, 2], F32, tag="gb")
            for t in range(NT):
                nc.sync.dma_start(out=gb_s[:, t, :], in_=gb_d[t])
            stats_acc = singles.tile([128, 2 * NT], F32, tag="stats_acc")
            stats_g = singles.tile([128, 2 * NT], F32, tag="stats_g")
            eps_t = singles.tile([128, 1], F32, tag="eps_t")
            nc.vector.memset(eps_t[:, :], BN_EPS)
            ones1 = singles.tile([1, 128], F32, tag="ones1")
            nc.vector.memset(ones1[:, :], 1.0)

            # ---- load x into padded layout ----
            xq = [[None] * NT for _ in range(B_LOC)]
            for i in range(B_LOC):
                for t in range(NT):
                    xt = xq_pool.tile([128, H, XW], F32R, tag=f"xq{i}{t}",
                                      name=f"xq{i}{t}", bufs=1)
                    nc.sync.dma_start(out=xt[:, :, :],
                                      in_=x_d[i, t * 128:(t + 1) * 128])
                    xq[i][t] = xt

            # ---- per-sample attention / gating chain ----
            chains = []
            for i in range(B_LOC):
                ch = {}
                gsum = sp.tile([128, NT], F32, tag="gsum")
                tmp64 = sp.tile([128, H], F32, tag="tmp64")
                for t in range(NT):
                    nc.vector.reduce_sum(tmp64[:, :],
                                         xq[i][t][:, :, 1:W + 1].bitcast(F32),
                                         axis=AX.X)
                    nc.vector.reduce_sum(gsum[:, t:t + 1], tmp64[:, :],
                                         axis=AX.X)
                # max over in2 (tile 1) for the LGA mlp
                vv = sp.tile([128, 2], F32, tag="vv")
                tmp64b = sp.tile([128, H], F32, tag="tmp64b")
                nc.vector.reduce_max(tmp64b[:, :],
                                     xq[i][1][:, :, 1:W + 1].bitcast(F32),
                                     axis=AX.X)
                nc.vector.reduce_max(vv[:, 0:1], tmp64b[:, :], axis=AX.X)
                nc.vector.tensor_scalar_mul(vv[:, 1:2], gsum[:, 1:2],
                                            1.0 / HW)

                # gap -> free layout (two (128,1) -> (1,128) transposes)
                gf = []
                for t in range(NT):
                    g_ps = pss.tile([1, 128], F32, tag="pst")
                    nc.tensor.transpose(g_ps[:, :], gsum[:, t:t + 1],
                                        ident[:, :])
                    gf.append(g_ps)
                g2 = sp.tile([1, C + 2], F32, tag="g2")
                nc.vector.memset(g2[:, :], 0.0)
                for t in range(NT):
                    nc.vector.tensor_copy(
                        out=g2[0:1, 1 + t * 128:1 + (t + 1) * 128],
                        in_=gf[t][0:1, :])
                gs = sp.tile([1, 130], F32, tag="gs")
                nc.vector.memset(gs[:, :], 0.0)
                nc.vector.tensor_copy(out=gs[0:1, 1:129], in_=gf[0][0:1, :])

                # t = conv1d(gap_mean, att_w) : weights pre-scaled by 1/HW
                ta = sp.tile([1, C], F32, tag="ta")
                tb = sp.tile([1, C], F32, tag="tb")
                t_t = sp.tile([1, C], F32, tag="t_t")
                nc.vector.tensor_scalar_mul(ta[:, :], g2[0:1, 0:C],
                                            smalls[0:1, 0:1])
                nc.vector.scalar_tensor_tensor(tb[:, :], g2[0:1, 1:C + 1],
                                               smalls[0:1, 1:2], ta[:, :],
                                               ALU.mult, ALU.add)
                nc.vector.scalar_tensor_tensor(t_t[:, :], g2[0:1, 2:C + 2],
                                               smalls[0:1, 2:3], tb[:, :],
                                               ALU.mult, ALU.add)

                # s = conv1d(gap1_mean, lga_w) + b (bias added in free layout)
                sa = sp.tile([1, 128], F32, tag="sa")
                sb = sp.tile([1, 128], F32, tag="sb")
                s_t = sp.tile([1, 128], F32, tag="s_t")
                nc.vector.tensor_scalar_mul(sa[:, :], gs[0:1, 0:128],
                                            smalls[0:1, 3:4])
                nc.vector.scalar_tensor_tensor(sb[:, :], gs[0:1, 1:129],
                                               smalls[0:1, 4:5], sa[:, :],
                                               ALU.mult, ALU.add)
                nc.vector.scalar_tensor_tensor(s_t[:, :], gs[0:1, 2:130],
                                               smalls[0:1, 5:6], sb[:, :],
                                               ALU.mult, ALU.add)
                nc.vector.tensor_scalar_add(s_t[:, :], s_t[:, :],
                                            smalls[0:1, 6:7])

                # transposes back to partition layout
                tps = sp.tile([128, NT], F32, tag="tps")
                ia = sp.tile([128, NT], F32, tag="ia")
                for t in range(NT):
                    tp_ps = pss.tile([128, 1], F32, tag="pst")
                    nc.tensor.transpose(tp_ps[:, :],
                                        t_t[0:1, t * 128:(t + 1) * 128],
                                        ident[0:1, 0:1])
                    nc.vector.tensor_copy(out=tps[:, t:t + 1],
                                          in_=tp_ps[:, :])
                    nc.scalar.activation(out=ia[:, t:t + 1], in_=tp_ps[:, :],
                                         func=ACT.Sigmoid)
                sk = sp.tile([128, 2], F32, tag="sk")
                sp_ps = pss.tile([128, 1], F32, tag="pst")
                nc.tensor.transpose(sp_ps[:, :], s_t[0:1, :],
                                    ident[0:1, 0:1])
                nc.scalar.activation(out=sk[:, 0:1], in_=sp_ps[:, :],
                                     func=ACT.Sigmoid)

                # out_att (permuted) per co tile
                oatt = sp.tile([128, NT], F32, tag="oatt")
                for ct in range(NT):
                    o_ps = pss.tile([128, 1], F32, tag="pst")
                    for t in range(NT):
                        nc.tensor.matmul(
                            o_ps[:, :],
                            aow_s[t][:, ct * 128:(ct + 1) * 128],
                            tps[:, t:t + 1],
                            start=(t == 0), stop=(t == NT - 1))
                    nc.scalar.activation(out=oatt[:, ct:ct + 1],
                                         in_=o_ps[:, :], func=ACT.Sigmoid)

                # kernel attention logits -> softmax -> broadcast
                kl_ps = pss.tile([KNUM, 1], F32, tag="pst")
                for t in range(NT):
                    nc.tensor.matmul(kl_ps[:, :], akw_s[t][:, :],
                                     tps[:, t:t + 1],
                                     start=(t == 0), stop=(t == NT - 1))
                kls = sp.tile([KNUM, 1], F32, tag="kls")
                nc.vector.tensor_copy(out=kls[:, :], in_=kl_ps[:, :])
                kt_ps = pss.tile([1, KNUM], F32, tag="pst")
                nc.tensor.transpose(kt_ps[:, :], kls[:, :],
                                    ident[0:KNUM, 0:KNUM])
                mx = sp.tile([1, 1], F32, tag="mx")
                nc.vector.reduce_max(mx[:, :], kt_ps[0:1, :], axis=AX.X)
                ex = sp.tile([1, KNUM], F32, tag="ex")
                nc.vector.tensor_scalar(out=ex[:, :], in0=kt_ps[0:1, :],
                                        scalar1=mx[:, :], scalar2=None,
                                        op0=ALU.subtract)
                exs = sp.tile([1, KNUM], F32, tag="exs")
                nc.scalar.activation(out=exs[:, :], in_=ex[:, :],
                                     func=ACT.Exp)
                sm1 = sp.tile([1, 1], F32, tag="sm1")
                nc.vector.reduce_sum(sm1[:, :], exs[:, :], axis=AX.X)
                nc.vector.reciprocal(out=sm1[:, :], in_=sm1[:, :])
                katt = sp.tile([1, KNUM], F32, tag="katt")
                nc.vector.tensor_scalar_mul(katt[:, :], exs[:, :],
                                            sm1[:, :])
                kattb = sp.tile([128, KNUM], F32, tag="kattb")
                kb_ps = pss.tile([128, KNUM], F32, tag="pst")
                nc.tensor.matmul(kb_ps[:, :], ones1[:, :], katt[0:1, :],
                                 start=True, stop=True)
                nc.vector.tensor_copy(out=kattb[:, :], in_=kb_ps[:, :])

                # LGA mlp: sigmoid(mlp(max) + mlp(mean))
                h_ps = pss.tile([16, 2], F32, tag="pst")
                nc.tensor.matmul(h_ps[:, :], w1t_s[:, :], vv[:, :],
                                 start=True, stop=True)
                h_s = sp.tile([16, 2], F32, tag="h_s")
                nc.scalar.activation(out=h_s[:, :], in_=h_ps[:, :],
                                     func=ACT.Relu)
                m_ps = pss.tile([128, 2], F32, tag="pst")
                nc.tensor.matmul(m_ps[:, :], w2t_s[:, :], h_s[:, :],
                                 start=True, stop=True)
                mcp = sp.tile([128, 2], F32, tag="mcp")
                nc.vector.tensor_copy(out=mcp[:, :], in_=m_ps[:, :])
                chadd = sp.tile([128, 1], F32, tag="chadd")
                nc.vector.tensor_add(chadd[:, :], mcp[:, 0:1], mcp[:, 1:2])
                nc.scalar.activation(out=sk[:, 1:2], in_=chadd[:, :],
                                     func=ACT.Sigmoid)

                ch["kattb"] = kattb
                ch["ia"] = ia
                ch["oatt"] = oatt
                ch["sk"] = sk
                chains.append(ch)

            # ---- build dynamic conv weights: agg = in_att * sum_k katt_k E_k
            agg = [[None] * NT for _ in range(B_LOC)]
            for i in range(B_LOC):
                for t in range(NT):
                    agg[i][t] = agg_pool.tile([128, EFREE], F32R,
                                              tag=f"agg{i}{t}",
                                              name=f"agg{i}{t}", bufs=1)
            for t in range(NT):
                for k in range(KNUM):
                    for hh in range(2):
                        et = e_pool.tile([128, EHALF], F32, tag="e")
                        nc.sync.dma_start(
                            out=et[:, :],
                            in_=e_d[k, t, :, hh * EHALF:(hh + 1) * EHALF])
                        for i in range(B_LOC):
                            dst = agg[i][t][:, hh * EHALF:(hh + 1) * EHALF]
                            kap = chains[i]["kattb"][:, k:k + 1]
                            if k == 0:
                                nc.vector.tensor_scalar_mul(dst, et[:, :],
                                                            kap)
                            else:
                                nc.vector.scalar_tensor_tensor(
                                    dst, et[:, :], kap, dst,
                                    ALU.mult, ALU.add)
                for i in range(B_LOC):
                    nc.vector.tensor_scalar_mul(
                        agg[i][t][:, :], agg[i][t][:, :],
                        chains[i]["ia"][:, t:t + 1])

            # ---- z init (K branch), conv, drains, stats ----
            z = [[None] * NT for _ in range(B_LOC)]
            for i in range(B_LOC):
                for t in range(NT):
                    zt = z_pool.tile([128, HW], F32, tag=f"z{i}{t}",
                                        name=f"z{i}{t}", bufs=1)
                    nc.vector.tensor_scalar_mul(
                        zt[:, :], xq[i][t][:, :, 1:W + 1].bitcast(F32),
                        chains[i]["sk"][:, t:t + 1])
                    z[i][t] = zt

            for i in range(B_LOC):
                for ct in range(NT):
                    for grp in range(2):
                        banks = []
                        for j in range(4):
                            banks.append(ps_conv.tile([128, 8, W], F32,
                                                      tag="cps",
                                                      name=f"cps{j}"))
                        for cit in range(NT):
                            for (dp, dq) in SHIFTS:
                                pq = (dp + 1) * 3 + (dq + 1)
                                lo = pq * C + ct * 128
                                lhs = agg[i][cit][:, lo:lo + 128]
                                for j in range(4):
                                    chunk = grp * 4 + j
                                    y0 = chunk * 8
                                    ylo = max(y0, -dp)
                                    yhi = min(y0 + 7, H - 1 - dp)
                                    n_r = yhi - ylo + 1
                                    if n_r <= 0:
                                        continue
                                    out_ap = banks[j][:, ylo - y0:
                                                      ylo - y0 + n_r, :]
                                    in_ap = xq[i][cit][:, ylo + dp:
                                                       ylo + dp + n_r,
                                                       1 + dq:1 + dq + W]
                                    first = (cit == 0 and dp == 0 and dq == 0)
                                    last = (cit == NT - 1
                                            and (dp, dq) == SHIFTS[-1])
                                    nc.tensor.matmul(out_ap, lhs, in_ap,
                                                     start=first, stop=last,
                                                     skip_group_check=True)
                        # drain: z = psum * out_att + z
                        for j in range(4):
                            chunk = grp * 4 + j
                            zsl = z[i][ct][:, chunk * 512:(chunk + 1) * 512]
                            nc.vector.scalar_tensor_tensor(
                                zsl, banks[j][:, :, :],
                                chains[i]["oatt"][:, ct:ct + 1], zsl,
                                ALU.mult, ALU.add)
                    # per (i, ct) batchnorm partial stats
                    st = sp.tile([128, 8, 6], F32, tag="bnst")
                    for j in range(8):
                        nc.vector.bn_stats(out=st[:, j, :],
                                           in_=z[i][ct][:, j * 512:
                                                        (j + 1) * 512])
                    mv = sp.tile([128, 2], F32, tag="mv")
                    nc.vector.bn_aggr(out=mv[:, :], in_=st[:, :, :])
                    m2 = sp.tile([128, 1], F32, tag="m2")
                    nc.vector.tensor_mul(m2[:, :], mv[:, 0:1], mv[:, 0:1])
                    ex2 = sp.tile([128, 1], F32, tag="ex2")
                    nc.vector.tensor_add(ex2[:, :], mv[:, 1:2], m2[:, :])
                    cs = 2 * ct
                    if i == 0:
                        nc.vector.tensor_scalar_mul(
                            stats_acc[:, cs:cs + 1], mv[:, 0:1], float(HW))
                        nc.vector.tensor_scalar_mul(
                            stats_acc[:, cs + 1:cs + 2], ex2[:, :],
                            float(HW))
                    else:
                        nc.vector.scalar_tensor_tensor(
                            stats_acc[:, cs:cs + 1], mv[:, 0:1], float(HW),
                            stats_acc[:, cs:cs + 1], ALU.mult, ALU.add)
                        nc.vector.scalar_tensor_tensor(
                            stats_acc[:, cs + 1:cs + 2], ex2[:, :],
                            float(HW), stats_acc[:, cs + 1:cs + 2],
                            ALU.mult, ALU.add)

            # ---- cross-core batchnorm reduction ----
            st_in = dram.tile([128, 2 * NT], F32, tag="st_in")
            st_out = dram.tile([128, 2 * NT], F32, tag="st_out")
            nc.gpsimd.dma_start(out=st_in[:, :], in_=stats_acc[:, :])
            nc.gpsimd.collective_compute(
                "AllReduce", ALU.add,
                replica_groups=[list(range(N_CORES))],
                ins=[st_in[:, :].opt()], outs=[st_out[:, :].opt()])
            nc.gpsimd.dma_start(out=stats_g[:, :], in_=st_out[:, :])

            # ---- finalize BN, relu, write out (permuted channels) ----
            out_view = out_d[:, :, :, :].rearrange(
                "b (cl cr) h w -> b cr cl (h w)", cr=4)
            n_total = float(B * HW)
            for t in range(NT):
                mean = sp.tile([128, 1], F32, tag="mean")
                ex2g = sp.tile([128, 1], F32, tag="ex2g")
                nc.vector.tensor_scalar_mul(mean[:, :],
                                            stats_g[:, 2 * t:2 * t + 1],
                                            1.0 / n_total)
                nc.vector.tensor_scalar_mul(ex2g[:, :],
                                            stats_g[:, 2 * t + 1:2 * t + 2],
                                            1.0 / n_total)
                m2g = sp.tile([128, 1], F32, tag="m2g")
                nc.vector.tensor_mul(m2g[:, :], mean[:, :], mean[:, :])
                var = sp.tile([128, 1], F32, tag="var")
                nc.vector.tensor_sub(var[:, :], ex2g[:, :], m2g[:, :])
                rstd = sp.tile([128, 1], F32, tag="rstd")
                nc.scalar.activation(out=rstd[:, :], in_=var[:, :],
                                     func=ACT.Sqrt, bias=eps_t[:, :])
                nc.vector.reciprocal(out=rstd[:, :], in_=rstd[:, :])
                scl = sp.tile([128, 1], F32, tag="scl")
                nc.vector.tensor_mul(scl[:, :], gb_s[:, t, 0:1], rstd[:, :])
                tmpb = sp.tile([128, 1], F32, tag="tmpb")
                nc.vector.tensor_mul(tmpb[:, :], mean[:, :], scl[:, :])
                bia = sp.tile([128, 1], F32, tag="bia")
                nc.vector.tensor_sub(bia[:, :], gb_s[:, t, 1:2], tmpb[:, :])
                for i in range(B_LOC):
                    nc.scalar.activation(out=z[i][t][:, :], in_=z[i][t][:, :],
                                         func=ACT.Relu, bias=bia[:, :],
                                         scale=scl[:, :])
                    for ph in range(2):
                        nc.sync.dma_start(
                            out=out_view[i, 2 * t + ph, :, :],
                            in_=z[i][t][ph * 64:(ph + 1) * 64, :])
    nc.finalize()
    return nc


def _host_prep(inputs):
    """Numpy-side weight re-layouts (all small except ede transpose)."""
    c = np.arange(C)
    pinv = (c % 64) * 4 + c // 64          # output-channel permutation
    ede = np.ascontiguousarray(inputs["ede_weight"], dtype=np.float32)
    ede_p = ede[:, pinv]                    # permute co axis
    # -> [k, ci, pq, co] so an SBUF agg tile is [ci_part, pq*256+co]
    e_host = np.ascontiguousarray(
        ede_p.transpose(0, 2, 3, 4, 1).reshape(KNUM, NT, 128, EFREE))
    aow = np.ascontiguousarray(
        inputs["att_out_w"][pinv].T.reshape(NT, 128, C), dtype=np.float32)
    akw = np.ascontiguousarray(
        inputs["att_kernel_w"].T.reshape(NT, 128, KNUM), dtype=np.float32)
    w1t = np.ascontiguousarray(inputs["lga_mlp_w1"].T, dtype=np.float32)
    w2t = np.ascontiguousarray(inputs["lga_mlp_w2"].T, dtype=np.float32)
    gb = np.stack([np.asarray(inputs["bn_gamma"])[pinv].reshape(NT, 128),
                   np.asarray(inputs["bn_beta"])[pinv].reshape(NT, 128)],
                  axis=-1).astype(np.float32)
    aw = np.asarray(inputs["att_conv1d_w"], dtype=np.float32) / HW
    lw = np.asarray(inputs["lga_conv1d_w"], dtype=np.float32) / HW
    lb = float(np.asarray(inputs["lga_conv1d_b"]).reshape(-1)[0])
    sm = np.array([[aw[0], aw[1], aw[2], lw[0], lw[1], lw[2], lb, 0.0]],
                  dtype=np.float32)
    return e_host, aow, akw, w1t, w2t, gb, sm


_CACHE = {}
last_results = None


def kernel(_trace=False, **inputs):
    global last_results
    x = np.asarray(inputs["x"], dtype=np.float32)
    xpad = np.zeros((B, C, H, XW), np.float32)
    xpad[:, :, :, 1:W + 1] = x
    xpad = np.ascontiguousarray(xpad)
    e_host, aow, akw, w1t, w2t, gb, sm = _host_prep(inputs)

    if "nc" not in _CACHE:
        _CACHE["nc"] = build_program()
    nc = _CACHE["nc"]

    shared = {"ew": e_host, "aow": aow, "akw": akw, "w1t": w1t,
              "w2t": w2t, "gb": gb, "sm": sm}
    in_maps = []
    for core in range(N_CORES):
        m = dict(shared)
        m["x"] = xpad[core * B_LOC:(core + 1) * B_LOC]
        in_maps.append(m)

    res = run_bass_kernel_spmd(nc, in_maps, list(range(N_CORES)),
                               trace=_trace)
    last_results = res
    out = np.concatenate([res.results[i]["out"] for i in range(N_CORES)],
                         axis=0)
    return out


# revision 15
# speedup vs baseline: 13883.0805x; 13883.0805x over previous
"""Trainium2 Bass kernel for the DEAM dense-CNN block.

Data-parallel over batch: 16 samples -> 8 cores x 2 samples.
Per sample: attention chain (GAP -> conv1d -> sigmoid/softmax heads),
dynamic per-sample 3x3 conv as 9 shifted matmuls (fp32r), LGA gating
branch, fused add + batch BN (cross-core AllReduce of per-channel
sums) + ReLU.

Channel shuffle is folded into host-side weight permutations plus a
permuted output DMA, so no on-chip data movement is spent on it.
"""

import numpy as np

import concourse.bass as bass
import concourse.mybir as mybir
import concourse.tile as tile
from concourse import bacc
from concourse.bass_utils import run_bass_kernel_spmd
from concourse.masks import make_identity

F32 = mybir.dt.float32
F32R = mybir.dt.float32r
AX = mybir.AxisListType
ALU = mybir.AluOpType
ACT = mybir.ActivationFunctionType

B, C, H, W = 16, 256, 64, 64
HW = H * W
KNUM, KS = 4, 3
N_CORES = 8
B_LOC = B // N_CORES          # 2 samples per core
NT = C // 128                 # 2 channel tiles
BN_EPS = 1e-5
XW = W + 2                    # padded row width 66
PQ = KS * KS                  # 9
EFREE = PQ * C                # 2304 free elems of an agg/E tile
EHALF = EFREE // 2            # 1152

# shift order: (0,0) first so the start=True matmul covers the full bank
SHIFTS = [(0, 0), (0, -1), (0, 1), (-1, -1), (-1, 0), (-1, 1),
          (1, -1), (1, 0), (1, 1)]


def build_program():
    nc = bacc.Bacc("TRN2", target_bir_lowering=False, debug=False,
                   num_devices=N_CORES)

    x_d = nc.dram_tensor("x", [B_LOC, C, H, XW], F32R,
                        kind="ExternalInput")
    e_d = nc.dram_tensor("ew", [KNUM, NT, 128, EFREE], F32,
                         kind="ExternalInput")
    aow_d = nc.dram_tensor("aow", [NT, 128, C], F32, kind="ExternalInput")
    akw_d = nc.dram_tensor("akw", [NT, 128, KNUM], F32, kind="ExternalInput")
    w1t_d = nc.dram_tensor("w1t", [128, 16], F32, kind="ExternalInput")
    w2t_d = nc.dram_tensor("w2t", [16, 128], F32, kind="ExternalInput")
    gb_d = nc.dram_tensor("gb", [NT, 128, 2], F32, kind="ExternalInput")
    sm_d = nc.dram_tensor("sm", [1, 8], F32, kind="ExternalInput")
    out_d = nc.dram_tensor("out", [B_LOC, C, H, W], F32,
                           kind="ExternalOutput")

    with tile.TileContext(nc) as tc:
        with (
            tc.tile_pool(name="singles", bufs=1) as singles,
            tc.tile_pool(name="xq", bufs=2 * B_LOC) as xq_pool,
            tc.tile_pool(name="z", bufs=2 * B_LOC) as z_pool,
            tc.tile_pool(name="ep", bufs=2) as e_pool,
            tc.tile_pool(name="agg", bufs=2 * B_LOC) as agg_pool,
            tc.tile_pool(name="sp", bufs=2) as sp,
            tc.tile_pool(name="psc", bufs=4, space="PSUM") as ps_conv,
            tc.tile_pool(name="pss", bufs=2, space="PSUM") as pss,
            tc.tile_pool(name="dram", bufs=2, space="DRAM") as dram,
        ):
            # ---- constants ----
            ident = singles.tile([128, 128], F32, tag="ident")
            make_identity(nc, ident[:, :])
            smalls = singles.tile([1, 8], F32, tag="smalls")
            nc.sync.dma_start(out=smalls[:, :], in_=sm_d[:, :])
            aow_s = []
            akw_s = []
            for t in range(NT):
                a = singles.tile([128, C], F32, tag=f"aow{t}")
                nc.sync.dma_start(out=a[:, :], in_=aow_d[t])
                aow_s.append(a)
                k = singles.tile([128, KNUM], F32, tag=f"akw{t}")
                nc.sync.dma_start(out=k[:, :], in_=akw_d[t])
                akw_s.append(k)
            w1t_s = singles.tile([128, 16], F32, tag="w1t")
            nc.sync.dma_start(out=w1t_s[:, :], in_=w1t_d[:, :])
            w2t_s = singles.tile([16, 128], F32, tag="w2t")
            nc.sync.dma_start(out=w2t_s[:, :], in_=w2t_d[:, :])
            gb_s = singles.tile([128, NT, 2], F32, tag="gb")
            for t in range(NT):
                nc.sync.dma_start(out=gb_s[:, t, :], in_=gb_d[t])
            stats_acc = singles.tile([128, 2 * NT], F32, tag="stats_acc")
            stats_g = singles.tile([128, 2 * NT], F32, tag="stats_g")
            eps_t = singles.tile([128, 1], F32, tag="eps_t")
            nc.vector.memset(eps_t[:, :], BN_EPS)
            ones1 = singles.tile([1, 128], F32, tag="ones1")
            nc.vector.memset(ones1[:, :], 1.0)

            # ---- load x into padded layout ----
            xq = [[None] * NT for _ in range(B_LOC)]
            for i in range(B_LOC):
                for t in range(NT):
                    xt = xq_pool.tile([128, H, XW], F32R, tag=f"xq{i}{t}",
                                      name=f"xq{i}{t}", bufs=1)
                    nc.sync.dma_start(out=xt[:, :, :],
                                      in_=x_d[i, t * 128:(t + 1) * 128])
                    xq[i][t] = xt

            # ---- per-sample attention / gating chain ----
            chains = []
            for i in range(B_LOC):
                ch = {}
                gsum = sp.tile([128, NT], F32, tag="gsum")
                tmp64 = sp.tile([128, H], F32, tag="tmp64")
                for t in range(NT):
                    nc.vector.reduce_sum(tmp64[:, :],
                                         xq[i][t][:, :, 1:W + 1].bitcast(F32),
                                         axis=AX.X)
                    nc.vector.reduce_sum(gsum[:, t:t + 1], tmp64[:, :],
                                         axis=AX.X)
                # max over in2 (tile 1) for the LGA mlp
                vv = sp.tile([128, 2], F32, tag="vv")
                tmp64b = sp.tile([128, H], F32, tag="tmp64b")
                nc.vector.reduce_max(tmp64b[:, :],
                                     xq[i][1][:, :, 1:W + 1].bitcast(F32),
                                     axis=AX.X)
                nc.vector.reduce_max(vv[:, 0:1], tmp64b[:, :], axis=AX.X)
                nc.vector.tensor_scalar_mul(vv[:, 1:2], gsum[:, 1:2],
                                            1.0 / HW)

                # gap -> free layout (two (128,1) -> (1,128) transposes)
                gf = []
                for t in range(NT):
                    g_ps = pss.tile([1, 128], F32, tag="pst")
                    nc.tensor.transpose(g_ps[:, :], gsum[:, t:t + 1],
                                        ident[:, :])
                    gf.append(g_ps)
                g2 = sp.tile([1, C + 2], F32, tag="g2")
                nc.vector.memset(g2[:, :], 0.0)
                for t in range(NT):
                    nc.vector.tensor_copy(
                        out=g2[0:1, 1 + t * 128:1 + (t + 1) * 128],
                        in_=gf[t][0:1, :])
                gs = sp.tile([1, 130], F32, tag="gs")
                nc.vector.memset(gs[:, :], 0.0)
                nc.vector.tensor_copy(out=gs[0:1, 1:129], in_=gf[0][0:1, :])

                # t = conv1d(gap_mean, att_w) : weights pre-scaled by 1/HW
                ta = sp.tile([1, C], F32, tag="ta")
                tb = sp.tile([1, C], F32, tag="tb")
                t_t = sp.tile([1, C], F32, tag="t_t")
                nc.vector.tensor_scalar_mul(ta[:, :], g2[0:1, 0:C],
                                            smalls[0:1, 0:1])
                nc.vector.scalar_tensor_tensor(tb[:, :], g2[0:1, 1:C + 1],
                                               smalls[0:1, 1:2], ta[:, :],
                                               ALU.mult, ALU.add)
                nc.vector.scalar_tensor_tensor(t_t[:, :], g2[0:1, 2:C + 2],
                                               smalls[0:1, 2:3], tb[:, :],
                                               ALU.mult, ALU.add)

                # s = conv1d(gap1_mean, lga_w) + b (bias added in free layout)
                sa = sp.tile([1, 128], F32, tag="sa")
                sb = sp.tile([1, 128], F32, tag="sb")
                s_t = sp.tile([1, 128], F32, tag="s_t")
                nc.vector.tensor_scalar_mul(sa[:, :], gs[0:1, 0:128],
                                            smalls[0:1, 3:4])
                nc.vector.scalar_tensor_tensor(sb[:, :], gs[0:1, 1:129],
                                               smalls[0:1, 4:5], sa[:, :],
                                               ALU.mult, ALU.add)
                nc.vector.scalar_tensor_tensor(s_t[:, :], gs[0:1, 2:130],
                                               smalls[0:1, 5:6], sb[:, :],
                                               ALU.mult, ALU.add)
                nc.vector.tensor_scalar_add(s_t[:, :], s_t[:, :],
                                            smalls[0:1, 6:7])

                # transposes back to partition layout
                tps = sp.tile([128, NT], F32, tag="tps")
                ia = sp.tile([128, NT], F32, tag="ia")
                for t in range(NT):
                    tp_ps = pss.tile([128, 1], F32, tag="pst")
                    nc.tensor.transpose(tp_ps[:, :],
                                        t_t[0:1, t * 128:(t + 1) * 128],
                                        ident[0:1, 0:1])
                    nc.vector.tensor_copy(out=tps[:, t:t + 1],
                                          in_=tp_ps[:, :])
                    nc.scalar.activation(out=ia[:, t:t + 1], in_=tp_ps[:, :],
                                         func=ACT.Sigmoid)
                sk = sp.tile([128, 2], F32, tag="sk")
                sp_ps = pss.tile([128, 1], F32, tag="pst")
                nc.tensor.transpose(sp_ps[:, :], s_t[0:1, :],
                                    ident[0:1, 0:1])
                nc.scalar.activation(out=sk[:, 0:1], in_=sp_ps[:, :],
                                     func=ACT.Sigmoid)

                # out_att (permuted) per co tile
                oatt = sp.tile([128, NT], F32, tag="oatt")
                for ct in range(NT):
                    o_ps = pss.tile([128, 1], F32, tag="pst")
                    for t in range(NT):
                        nc.tensor.matmul(
                            o_ps[:, :],
                            aow_s[t][:, ct * 128:(ct + 1) * 128],
                            tps[:, t:t + 1],
                            start=(t == 0), stop=(t == NT - 1))
                    nc.scalar.activation(out=oatt[:, ct:ct + 1],
                                         in_=o_ps[:, :], func=ACT.Sigmoid)

                # kernel attention logits -> softmax -> broadcast
                kl_ps = pss.tile([KNUM, 1], F32, tag="pst")
                for t in range(NT):
                    nc.tensor.matmul(kl_ps[:, :], akw_s[t][:, :],
                                     tps[:, t:t + 1],
                                     start=(t == 0), stop=(t == NT - 1))
                kls = sp.tile([KNUM, 1], F32, tag="kls")
                nc.vector.tensor_copy(out=kls[:, :], in_=kl_ps[:, :])
                kt_ps = pss.tile([1, KNUM], F32, tag="pst")
                nc.tensor.transpose(kt_ps[:, :], kls[:, :],
                                    ident[0:KNUM, 0:KNUM])
                mx = sp.tile([1, 1], F32, tag="mx")
                nc.vector.reduce_max(mx[:, :], kt_ps[0:1, :], axis=AX.X)
                ex = sp.tile([1, KNUM], F32, tag="ex")
                nc.vector.tensor_scalar(out=ex[:, :], in0=kt_ps[0:1, :],
                                        scalar1=mx[:, :], scalar2=None,
                                        op0=ALU.subtract)
                exs = sp.tile([1, KNUM], F32, tag="exs")
                nc.scalar.activation(out=exs[:, :], in_=ex[:, :],
                                     func=ACT.Exp)
                sm1 = sp.tile([1, 1], F32, tag="sm1")
                nc.vector.reduce_sum(sm1[:, :], exs[:, :], axis=AX.X)
                nc.vector.reciprocal(out=sm1[:, :], in_=sm1[:, :])
                katt = sp.tile([1, KNUM], F32, tag="katt")
                nc.vector.tensor_scalar_mul(katt[:, :], exs[:, :],
                                            sm1[:, :])
                kattb = sp.tile([128, KNUM], F32, tag="kattb")
                kb_ps = pss.tile([128, KNUM], F32, tag="pst")
                nc.tensor.matmul(kb_ps[:, :], ones1[:, :], katt[0:1, :],
                                 start=True, stop=True)
                nc.vector.tensor_copy(out=kattb[:, :], in_=kb_ps[:, :])

                # LGA mlp: sigmoid(mlp(max) + mlp(mean))
                h_ps = pss.tile([16, 2], F32, tag="pst")
                nc.tensor.matmul(h_ps[:, :], w1t_s[:, :], vv[:, :],
                                 start=True, stop=True)
                h_s = sp.tile([16, 2], F32, tag="h_s")
                nc.scalar.activation(out=h_s[:, :], in_=h_ps[:, :],
                                     func=ACT.Relu)
                m_ps = pss.tile([128, 2], F32, tag="pst")
                nc.tensor.matmul(m_ps[:, :], w2t_s[:, :], h_s[:, :],
                                 start=True, stop=True)
                mcp = sp.tile([128, 2], F32, tag="mcp")
                nc.vector.tensor_copy(out=mcp[:, :], in_=m_ps[:, :])
                chadd = sp.tile([128, 1], F32, tag="chadd")
                nc.vector.tensor_add(chadd[:, :], mcp[:, 0:1], mcp[:, 1:2])
                nc.scalar.activation(out=sk[:, 1:2], in_=chadd[:, :],
                                     func=ACT.Sigmoid)

                ch["kattb"] = kattb
                ch["ia"] = ia
                ch["oatt"] = oatt
                ch["sk"] = sk
                chains.append(ch)

            # ---- build dynamic conv weights: agg = in_att * sum_k katt_k E_k
            agg = [[None] * NT for _ in range(B_LOC)]
            for i in range(B_LOC):
                for t in range(NT):
                    agg[i][t] = agg_pool.tile([128, EFREE], F32R,
                                              tag=f"agg{i}{t}",
                                              name=f"agg{i}{t}", bufs=1)
            for t in range(NT):
                for k in range(KNUM):
                    for hh in range(2):
                        et = e_pool.tile([128, EHALF], F32, tag="e")
                        nc.sync.dma_start(
                            out=et[:, :],
                            in_=e_d[k, t, :, hh * EHALF:(hh + 1) * EHALF])
                        for i in range(B_LOC):
                            dst = agg[i][t][:, hh * EHALF:(hh + 1) * EHALF]
                            kap = chains[i]["kattb"][:, k:k + 1]
                            if k == 0:
                                nc.vector.tensor_scalar_mul(dst, et[:, :],
                                                            kap)
                            else:
                                nc.vector.scalar_tensor_tensor(
                                    dst, et[:, :], kap, dst,
                                    ALU.mult, ALU.add)
                for i in range(B_LOC):
                    nc.vector.tensor_scalar_mul(
                        agg[i][t][:, :], agg[i][t][:, :],
                        chains[i]["ia"][:, t:t + 1])

            # ---- z init (K branch), conv, drains, stats ----
            z = [[None] * NT for _ in range(B_LOC)]
            for i in range(B_LOC):
                for t in range(NT):
                    zt = z_pool.tile([128, HW], F32, tag=f"z{i}{t}",
                                        name=f"z{i}{t}", bufs=1)
                    nc.vector.tensor_scalar_mul(
                        zt[:, :], xq[i][t][:, :, 1:W + 1].bitcast(F32),
                        chains[i]["sk"][:, t:t + 1])
                    z[i][t] = zt

            for i in range(B_LOC):
                for ct in range(NT):
                    for grp in range(2):
                        banks = []
                        for j in range(4):
                            banks.append(ps_conv.tile([128, 8, W], F32,
                                                      tag="cps",
                                                      name=f"cps{j}"))
                        for cit in range(NT):
                            for (dp, dq) in SHIFTS:
                                pq = (dp + 1) * 3 + (dq + 1)
                                lo = pq * C + ct * 128
                                lhs = agg[i][cit][:, lo:lo + 128]
                                for j in range(4):
                                    chunk = grp * 4 + j
                                    y0 = chunk * 8
                                    ylo = max(y0, -dp)
                                    yhi = min(y0 + 7, H - 1 - dp)
                                    n_r = yhi - ylo + 1
                                    if n_r <= 0:
                                        continue
                                    out_ap = banks[j][:, ylo - y0:
                                                      ylo - y0 + n_r, :]
                                    in_ap = xq[i][cit][:, ylo + dp:
                                                       ylo + dp + n_r,
                                                       1 + dq:1 + dq + W]
                                    first = (cit == 0 and dp == 0 and dq == 0)
                                    last = (cit == NT - 1
                                            and (dp, dq) == SHIFTS[-1])
                                    nc.tensor.matmul(out_ap, lhs, in_ap,
                                                     start=first, stop=last,
                                                     skip_group_check=True)
                        # drain: z = psum * out_att + z
                        for j in range(4):
                            chunk = grp * 4 + j
                            zsl = z[i][ct][:, chunk * 512:(chunk + 1) * 512]
                            nc.vector.scalar_tensor_tensor(
                                zsl, banks[j][:, :, :],
                                chains[i]["oatt"][:, ct:ct + 1], zsl,
                                ALU.mult, ALU.add)
                    # per (i, ct) batchnorm partial stats
                    st = sp.tile([128, 8, 6], F32, tag="bnst")
                    for j in range(8):
                        nc.vector.bn_stats(out=st[:, j, :],
                                           in_=z[i][ct][:, j * 512:
                                                        (j + 1) * 512])
                    mv = sp.tile([128, 2], F32, tag="mv")
                    nc.vector.bn_aggr(out=mv[:, :], in_=st[:, :, :])
                    m2 = sp.tile([128, 1], F32, tag="m2")
                    nc.vector.tensor_mul(m2[:, :], mv[:, 0:1], mv[:, 0:1])
                    ex2 = sp.tile([128, 1], F32, tag="ex2")
                    nc.vector.tensor_add(ex2[:, :], mv[:, 1:2], m2[:, :])
                    cs = 2 * ct
                    if i == 0:
                        nc.vector.tensor_scalar_mul(
                            stats_acc[:, cs:cs + 1], mv[:, 0:1], float(HW))
                        nc.vector.tensor_scalar_mul(
                            stats_acc[:, cs + 1:cs + 2], ex2[:, :],
                            float(HW))
                    else:
                        nc.vector.scalar_tensor_tensor(
                            stats_acc[:, cs:cs + 1], mv[:, 0:1], float(HW),
                            stats_acc[:, cs:cs + 1], ALU.mult, ALU.add)
                        nc.vector.scalar_tensor_tensor(
                            stats_acc[:, cs + 1:cs + 2], ex2[:, :],
                            float(HW), stats_acc[:, cs + 1:cs + 2],
                            ALU.mult, ALU.add)

            # ---- cross-core batchnorm reduction ----
            st_in = dram.tile([128, 2 * NT], F32, tag="st_in")
            st_out = dram.tile([128, 2 * NT], F32, tag="st_out")
            nc.gpsimd.dma_start(out=st_in[:, :], in_=stats_acc[:, :])
            nc.gpsimd.collective_compute(
                "AllReduce", ALU.add,
                replica_groups=[list(range(N_CORES))],
                ins=[st_in[:, :].opt()], outs=[st_out[:, :].opt()])
            nc.gpsimd.dma_start(out=stats_g[:, :], in_=st_out[:, :])

            # ---- finalize BN, relu, write out (permuted channels) ----
            out_view = out_d[:, :, :, :].rearrange(
                "b (cl cr) h w -> b cr cl (h w)", cr=4)
            n_total = float(B * HW)
            for t in range(NT):
                mean = sp.tile([128, 1], F32, tag="mean")
                ex2g = sp.tile([128, 1], F32, tag="ex2g")
                nc.vector.tensor_scalar_mul(mean[:, :],
                                            stats_g[:, 2 * t:2 * t + 1],
                                            1.0 / n_total)
                nc.vector.tensor_scalar_mul(ex2g[:, :],
                                            stats_g[:, 2 * t + 1:2 * t + 2],
                                            1.0 / n_total)
                m2g = sp.tile([128, 1], F32, tag="m2g")
                nc.vector.tensor_mul(m2g[:, :], mean[:, :], mean[:, :])
                var = sp.tile([128, 1], F32, tag="var")
                nc.vector.tensor_sub(var[:, :], ex2g[:, :], m2g[:, :])
                rstd = sp.tile([128, 1], F32, tag="rstd")
                nc.scalar.activation(out=rstd[:, :], in_=var[:, :],
                                     func=ACT.Sqrt, bias=eps_t[:, :])
                nc.vector.reciprocal(out=rstd[:, :], in_=rstd[:, :])
                scl = sp.tile([128, 1], F32, tag="scl")
                nc.vector.tensor_mul(scl[:, :], gb_s[:, t, 0:1], rstd[:, :])
                tmpb = sp.tile([128, 1], F32, tag="tmpb")
                nc.vector.tensor_mul(tmpb[:, :], mean[:, :], scl[:, :])
                bia = sp.tile([128, 1], F32, tag="bia")
                nc.vector.tensor_sub(bia[:, :], gb_s[:, t, 1:2], tmpb[:, :])
                for i in range(B_LOC):
                    nc.scalar.activation(out=z[i][t][:, :], in_=z[i][t][:, :],
                                         func=ACT.Relu, bias=bia[:, :],
                                         scale=scl[:, :])
                    for ph in range(2):
                        nc.sync.dma_start(
                            out=out_view[i, 2 * t + ph, :, :],
                            in_=z[i][t][ph * 64:(ph + 1) * 64, :])
    nc.finalize()
    return nc


def _host_prep(inputs):
    """Numpy-side weight re-layouts (all small except ede transpose)."""
    c = np.arange(C)
    pinv = (c % 64) * 4 + c // 64          # output-channel permutation
    ede = np.ascontiguousarray(inputs["ede_weight"], dtype=np.float32)
    ede_p = ede[:, pinv]                    # permute co axis
    # -> [k, ci, pq, co] so an SBUF agg tile is [ci_part, pq*256+co]
    e_host = np.ascontiguousarray(
        ede_p.transpose(0, 2, 3, 4, 1).reshape(KNUM, NT, 128, EFREE))
    aow = np.ascontiguousarray(
        inputs["att_out_w"][pinv].T.reshape(NT, 128, C), dtype=np.float32)
    akw = np.ascontiguousarray(
        inputs["att_kernel_w"].T.reshape(NT, 128, KNUM), dtype=np.float32)
    w1t = np.ascontiguousarray(inputs["lga_mlp_w1"].T, dtype=np.float32)
    w2t = np.ascontiguousarray(inputs["lga_mlp_w2"].T, dtype=np.float32)
    gb = np.stack([np.asarray(inputs["bn_gamma"])[pinv].reshape(NT, 128),
                   np.asarray(inputs["bn_beta"])[pinv].reshape(NT, 128)],
                  axis=-1).astype(np.float32)
    aw = np.asarray(inputs["att_conv1d_w"], dtype=np.float32) / HW
    lw = np.asarray(inputs["lga_conv1d_w"], dtype=np.float32) / HW
    lb = float(np.asarray(inputs["lga_conv1d_b"]).reshape(-1)[0])
    sm = np.array([[aw[0], aw[1], aw[2], lw[0], lw[1], lw[2], lb, 0.0]],
                  dtype=np.float32)
    return e_host, aow, akw, w1t, w2t, gb, sm


_CACHE = {}
last_results = None


def _enable_axon_trace():
    """Register the NTFF profile hook that the agent image leaves out."""
    import sys
    import types

    import concourse.bass_utils as bu
    if "antenv.axon_hooks" in sys.modules:
        return
    from trn_agent_boot.trn_boot import _ntff_profile_via_ctypes
    hook = _ntff_profile_via_ctypes("/opt/axon/libaxon_pjrt.so")
    mod = types.ModuleType("antenv.axon_hooks")
    mod.get_axon_ntff_profile_hook = lambda: hook
    mod.set_axon_ntff_profile_hook = lambda h: None
    sys.modules["antenv.axon_hooks"] = mod
    bu.upload_artifacts = lambda tmpdir: f"local:{tmpdir}"


def kernel(_trace=False, _tmpdir=None, **inputs):
    global last_results
    if _trace:
        _enable_axon_trace()
    x = np.asarray(inputs["x"], dtype=np.float32)
    xpad = np.zeros((B, C, H, XW), np.float32)
    xpad[:, :, :, 1:W + 1] = x
    xpad = np.ascontiguousarray(xpad)
    e_host, aow, akw, w1t, w2t, gb, sm = _host_prep(inputs)

    if "nc" not in _CACHE:
        _CACHE["nc"] = build_program()
    nc = _CACHE["nc"]

    shared = {"ew": e_host, "aow": aow, "akw": akw, "w1t": w1t,
              "w2t": w2t, "gb": gb, "sm": sm}
    in_maps = []
    for core in range(N_CORES):
        m = dict(shared)
        m["x"] = xpad[core * B_LOC:(core + 1) * B_LOC]
        in_maps.append(m)

    res = run_bass_kernel_spmd(nc, in_maps, list(range(N_CORES)),
                               trace=_trace, tmpdir=_tmpdir)
    last_results = res
    out = np.concatenate([res.results[i]["out"] for i in range(N_CORES)],
                         axis=0)
    return out


# revision 18
# speedup vs baseline: 16959.4753x; 1.2216x over previous
"""Trainium2 Bass kernel for the DEAM dense-CNN block.

Data-parallel over batch: 16 samples -> 8 cores x 2 samples.
Per sample: attention chain (GAP -> conv1d -> sigmoid/softmax heads),
dynamic per-sample 3x3 conv as 9 shifted matmuls, LGA gating branch,
fused add + batch BN (cross-core AllReduce of per-channel sums) + ReLU.

Channel shuffle is folded into host-side weight permutations plus a
permuted output DMA, so no on-chip data movement is spent on it.

v2: conv runs in bf16 (fp32 streams at ~2 cycles/row on the PE; bf16
about 2x faster). Dynamic-weight build + GAP/max reductions run on
GpSimd so the Vector engine keeps up with PSUM drains. Per-sample BN
AllReduce: sample 0's reduction hides under sample 1's conv.
"""

import numpy as np
import ml_dtypes

import concourse.bass as bass
import concourse.mybir as mybir
import concourse.tile as tile
from concourse import bacc
from concourse.bass_utils import run_bass_kernel_spmd
from concourse.masks import make_identity

F32 = mybir.dt.float32
F32R = mybir.dt.float32r
BF16 = mybir.dt.bfloat16
AX = mybir.AxisListType
ALU = mybir.AluOpType
ACT = mybir.ActivationFunctionType

B, C, H, W = 16, 256, 64, 64
HW = H * W
KNUM, KS = 4, 3
N_CORES = 8
B_LOC = B // N_CORES          # 2 samples per core
NT = C // 128                 # 2 channel tiles
BN_EPS = 1e-5
XW = W + 2                    # padded row width 66
PQ = KS * KS                  # 9
EFREE = PQ * C                # 2304 free elems of an agg/E tile
EHALF = EFREE // 2            # 1152

CONV_BF16 = True              # bf16 conv (2x PE throughput) vs fp32r

# shift order: (0,0) first so the start=True matmul covers the full bank
SHIFTS = [(0, 0), (0, -1), (0, 1), (-1, -1), (-1, 0), (-1, 1),
          (1, -1), (1, 0), (1, 1)]


def build_program():
    cdt = BF16 if CONV_BF16 else F32R
    xdt = F32 if CONV_BF16 else F32R
    edt = BF16 if CONV_BF16 else F32

    nc = bacc.Bacc("TRN2", target_bir_lowering=False, debug=False,
                   num_devices=N_CORES)

    x_d = nc.dram_tensor("x", [B_LOC, C, H, XW], xdt, kind="ExternalInput")
    e_d = nc.dram_tensor("ew", [KNUM, NT, 128, EFREE], edt,
                         kind="ExternalInput")
    aow_d = nc.dram_tensor("aow", [NT, 128, C], F32, kind="ExternalInput")
    akw_d = nc.dram_tensor("akw", [NT, 128, KNUM], F32, kind="ExternalInput")
    w1t_d = nc.dram_tensor("w1t", [128, 16], F32, kind="ExternalInput")
    w2t_d = nc.dram_tensor("w2t", [16, 128], F32, kind="ExternalInput")
    gb_d = nc.dram_tensor("gb", [NT, 128, 2], F32, kind="ExternalInput")
    sm_d = nc.dram_tensor("sm", [1, 8], F32, kind="ExternalInput")
    out_d = nc.dram_tensor("out", [B_LOC, C, H, W], F32,
                           kind="ExternalOutput")

    with tile.TileContext(nc) as tc:
        with (
            tc.tile_pool(name="singles", bufs=1) as singles,
            tc.tile_pool(name="xq", bufs=1) as xq_pool,
            tc.tile_pool(name="z", bufs=1) as z_pool,
            tc.tile_pool(name="ep", bufs=4) as e_pool,
            tc.tile_pool(name="aggb", bufs=1) as aggb_pool,
            tc.tile_pool(name="sp", bufs=2) as sp,
            tc.tile_pool(name="psc", bufs=6, space="PSUM") as ps_conv,
            tc.tile_pool(name="pss", bufs=2, space="PSUM") as pss,
            tc.tile_pool(name="dram", bufs=4, space="DRAM") as dram,
        ):
            # ---- constants ----
            ident = singles.tile([128, 128], F32, tag="ident")
            make_identity(nc, ident[:, :])
            smalls = singles.tile([1, 8], F32, tag="smalls")
            nc.sync.dma_start(out=smalls[:, :], in_=sm_d[:, :])
            aow_s = []
            akw_s = []
            for t in range(NT):
                a = singles.tile([128, C], F32, tag=f"aow{t}")
                nc.sync.dma_start(out=a[:, :], in_=aow_d[t])
                aow_s.append(a)
                k = singles.tile([128, KNUM], F32, tag=f"akw{t}")
                nc.sync.dma_start(out=k[:, :], in_=akw_d[t])
                akw_s.append(k)
            w1t_s = singles.tile([128, 16], F32, tag="w1t")
            nc.sync.dma_start(out=w1t_s[:, :], in_=w1t_d[:, :])
            w2t_s = singles.tile([16, 128], F32, tag="w2t")
            nc.sync.dma_start(out=w2t_s[:, :], in_=w2t_d[:, :])
            gb_s = singles.tile([128, NT, 2], F32, tag="gb")
            for t in range(NT):
                nc.sync.dma_start(out=gb_s[:, t, :], in_=gb_d[t])
            eps_t = singles.tile([128, 1], F32, tag="eps_t")
            nc.vector.memset(eps_t[:, :], BN_EPS)
            ones1 = singles.tile([1, 128], F32, tag="ones1")
            nc.vector.memset(ones1[:, :], 1.0)
            # per-sample stats (sum, sumsq) x (tile0, tile1) + AR results
            sa = [singles.tile([128, 2 * NT], F32, tag=f"sa{i}",
                               name=f"sa{i}") for i in range(B_LOC)]
            sg = [singles.tile([128, 2 * NT], F32, tag=f"sg{i}",
                               name=f"sg{i}") for i in range(B_LOC)]

            # ---- load x (cast to bf16 via SW-DGE when CONV_BF16) ----
            dma_x = nc.gpsimd if CONV_BF16 else nc.sync
            xq = [[None] * NT for _ in range(B_LOC)]
            for i in range(B_LOC):
                for t in range(NT):
                    xt = xq_pool.tile([128, H, XW], cdt, tag=f"xq{i}{t}",
                                      name=f"xq{i}{t}", bufs=1)
                    dma_x.dma_start(out=xt[:, :, :],
                                    in_=x_d[i, t * 128:(t + 1) * 128])
                    xq[i][t] = xt

            def rd(ap):
                return ap if CONV_BF16 else ap.bitcast(F32)

            # ---- per-sample attention / gating chain ----
            chains = []
            for i in range(B_LOC):
                ch = {}
                gsum = sp.tile([128, NT], F32, tag="gsum")
                tmp64 = sp.tile([128, H], F32, tag="tmp64")
                for t in range(NT):
                    nc.vector.tensor_reduce(tmp64[:, :],
                                            rd(xq[i][t][:, :, 1:W + 1]),
                                            axis=AX.X, op=ALU.add)
                    nc.vector.tensor_reduce(gsum[:, t:t + 1], tmp64[:, :],
                                            axis=AX.X, op=ALU.add)
                # max over in2 (tile 1) for the LGA mlp
                vv = sp.tile([128, 2], F32, tag="vv")
                tmp64b = sp.tile([128, H], F32, tag="tmp64b")
                nc.vector.tensor_reduce(tmp64b[:, :],
                                        rd(xq[i][1][:, :, 1:W + 1]),
                                        axis=AX.X, op=ALU.max)
                nc.vector.tensor_reduce(vv[:, 0:1], tmp64b[:, :],
                                        axis=AX.X, op=ALU.max)
                nc.vector.tensor_scalar_mul(vv[:, 1:2], gsum[:, 1:2],
                                            1.0 / HW)

                # gap -> free layout (two (128,1) -> (1,128) transposes)
                gf = []
                for t in range(NT):
                    g_ps = pss.tile([1, 128], F32, tag="pst")
                    nc.tensor.transpose(g_ps[:, :], gsum[:, t:t + 1],
                                        ident[:, :])
                    gf.append(g_ps)
                g2 = sp.tile([1, C + 2], F32, tag="g2")
                nc.vector.memset(g2[:, :], 0.0)
                for t in range(NT):
                    nc.vector.tensor_copy(
                        out=g2[0:1, 1 + t * 128:1 + (t + 1) * 128],
                        in_=gf[t][0:1, :])
                gs = sp.tile([1, 130], F32, tag="gs")
                nc.vector.memset(gs[:, :], 0.0)
                nc.vector.tensor_copy(out=gs[0:1, 1:129], in_=gf[0][0:1, :])

                # t = conv1d(gap_mean, att_w) : weights pre-scaled by 1/HW
                ta = sp.tile([1, C], F32, tag="ta")
                tb = sp.tile([1, C], F32, tag="tb")
                t_t = sp.tile([1, C], F32, tag="t_t")
                nc.vector.tensor_scalar_mul(ta[:, :], g2[0:1, 0:C],
                                            smalls[0:1, 0:1])
                nc.vector.scalar_tensor_tensor(tb[:, :], g2[0:1, 1:C + 1],
                                               smalls[0:1, 1:2], ta[:, :],
                                               ALU.mult, ALU.add)
                nc.vector.scalar_tensor_tensor(t_t[:, :], g2[0:1, 2:C + 2],
                                               smalls[0:1, 2:3], tb[:, :],
                                               ALU.mult, ALU.add)

                # s = conv1d(gap1_mean, lga_w) + b (bias added in free layout)
                sa_ = sp.tile([1, 128], F32, tag="sa_")
                sb_ = sp.tile([1, 128], F32, tag="sb_")
                s_t = sp.tile([1, 128], F32, tag="s_t")
                nc.vector.tensor_scalar_mul(sa_[:, :], gs[0:1, 0:128],
                                            smalls[0:1, 3:4])
                nc.vector.scalar_tensor_tensor(sb_[:, :], gs[0:1, 1:129],
                                               smalls[0:1, 4:5], sa_[:, :],
                                               ALU.mult, ALU.add)
                nc.vector.scalar_tensor_tensor(s_t[:, :], gs[0:1, 2:130],
                                               smalls[0:1, 5:6], sb_[:, :],
                                               ALU.mult, ALU.add)
                nc.vector.tensor_scalar_add(s_t[:, :], s_t[:, :],
                                            smalls[0:1, 6:7])

                # transposes back to partition layout
                tps = sp.tile([128, NT], F32, tag="tps")
                ia = sp.tile([128, NT], F32, tag="ia")
                for t in range(NT):
                    tp_ps = pss.tile([128, 1], F32, tag="pst")
                    nc.tensor.transpose(tp_ps[:, :],
                                        t_t[0:1, t * 128:(t + 1) * 128],
                                        ident[0:1, 0:1])
                    nc.vector.tensor_copy(out=tps[:, t:t + 1],
                                          in_=tp_ps[:, :])
                    nc.scalar.activation(out=ia[:, t:t + 1], in_=tp_ps[:, :],
                                         func=ACT.Sigmoid)
                sk = sp.tile([128, 2], F32, tag="sk")
                sp_ps = pss.tile([128, 1], F32, tag="pst")
                nc.tensor.transpose(sp_ps[:, :], s_t[0:1, :],
                                    ident[0:1, 0:1])
                nc.scalar.activation(out=sk[:, 0:1], in_=sp_ps[:, :],
                                     func=ACT.Sigmoid)

                # out_att (permuted) per co tile
                oatt = sp.tile([128, NT], F32, tag="oatt")
                for ct in range(NT):
                    o_ps = pss.tile([128, 1], F32, tag="pst")
                    for t in range(NT):
                        nc.tensor.matmul(
                            o_ps[:, :],
                            aow_s[t][:, ct * 128:(ct + 1) * 128],
                            tps[:, t:t + 1],
                            start=(t == 0), stop=(t == NT - 1))
                    nc.scalar.activation(out=oatt[:, ct:ct + 1],
                                         in_=o_ps[:, :], func=ACT.Sigmoid)

                # kernel attention logits -> softmax -> broadcast
                kl_ps = pss.tile([KNUM, 1], F32, tag="pst")
                for t in range(NT):
                    nc.tensor.matmul(kl_ps[:, :], akw_s[t][:, :],
                                     tps[:, t:t + 1],
                                     start=(t == 0), stop=(t == NT - 1))
                kls = sp.tile([KNUM, 1], F32, tag="kls")
                nc.vector.tensor_copy(out=kls[:, :], in_=kl_ps[:, :])
                kt_ps = pss.tile([1, KNUM], F32, tag="pst")
                nc.tensor.transpose(kt_ps[:, :], kls[:, :],
                                    ident[0:KNUM, 0:KNUM])
                mx = sp.tile([1, 1], F32, tag="mx")
                nc.vector.reduce_max(mx[:, :], kt_ps[0:1, :], axis=AX.X)
                ex = sp.tile([1, KNUM], F32, tag="ex")
                nc.vector.tensor_scalar(out=ex[:, :], in0=kt_ps[0:1, :],
                                        scalar1=mx[:, :], scalar2=None,
                                        op0=ALU.subtract)
                exs = sp.tile([1, KNUM], F32, tag="exs")
                nc.scalar.activation(out=exs[:, :], in_=ex[:, :],
                                     func=ACT.Exp)
                sm1 = sp.tile([1, 1], F32, tag="sm1")
                nc.vector.reduce_sum(sm1[:, :], exs[:, :], axis=AX.X)
                nc.vector.reciprocal(out=sm1[:, :], in_=sm1[:, :])
                katt = sp.tile([1, KNUM], F32, tag="katt")
                nc.vector.tensor_scalar_mul(katt[:, :], exs[:, :],
                                            sm1[:, :])
                kattb = sp.tile([128, KNUM], F32, tag="kattb")
                kb_ps = pss.tile([128, KNUM], F32, tag="pst")
                nc.tensor.matmul(kb_ps[:, :], ones1[:, :], katt[0:1, :],
                                 start=True, stop=True)
                nc.vector.tensor_copy(out=kattb[:, :], in_=kb_ps[:, :])

                # LGA mlp: sigmoid(mlp(max) + mlp(mean))
                h_ps = pss.tile([16, 2], F32, tag="pst")
                nc.tensor.matmul(h_ps[:, :], w1t_s[:, :], vv[:, :],
                                 start=True, stop=True)
                h_s = sp.tile([16, 2], F32, tag="h_s")
                nc.scalar.activation(out=h_s[:, :], in_=h_ps[:, :],
                                     func=ACT.Relu)
                m_ps = pss.tile([128, 2], F32, tag="pst")
                nc.tensor.matmul(m_ps[:, :], w2t_s[:, :], h_s[:, :],
                                 start=True, stop=True)
                mcp = sp.tile([128, 2], F32, tag="mcp")
                nc.vector.tensor_copy(out=mcp[:, :], in_=m_ps[:, :])
                chadd = sp.tile([128, 1], F32, tag="chadd")
                nc.vector.tensor_add(chadd[:, :], mcp[:, 0:1], mcp[:, 1:2])
                nc.scalar.activation(out=sk[:, 1:2], in_=chadd[:, :],
                                     func=ACT.Sigmoid)

                # fold in_att into the softmax scalars: kia[ci,k] = katt_k*ia_t[ci]
                kia = sp.tile([128, NT, KNUM], F32, tag="kia")
                for t in range(NT):
                    nc.vector.tensor_scalar_mul(kia[:, t, :], kattb[:, :],
                                                ia[:, t:t + 1])
                ch["kia"] = kia
                ch["kattb"] = kattb
                ch["ia"] = ia
                ch["oatt"] = oatt
                ch["sk"] = sk
                chains.append(ch)

            # ---- dynamic conv weights: aggb = bf16(sum_k kia_k E_k)
            # (in_att folded into kia); E streamed once, MACs on DVE
            aggb = [[None] * NT for _ in range(B_LOC)]
            for i in range(B_LOC):
                for t in range(NT):
                    aggb[i][t] = aggb_pool.tile(
                        [128, EFREE], cdt, tag=f"aggb{i}{t}",
                        name=f"aggb{i}{t}", bufs=1)
            for t in range(NT):
                for k in range(KNUM):
                    for hh in range(2):
                        et = e_pool.tile([128, EHALF], edt, tag="e")
                        nc.sync.dma_start(
                            out=et[:, :],
                            in_=e_d[k, t, :, hh * EHALF:(hh + 1) * EHALF])
                        for i in range(B_LOC):
                            dst = aggb[i][t][:, hh * EHALF:(hh + 1) * EHALF]
                            kap = chains[i]["kia"][:, t, k:k + 1]
                            if k == 0:
                                nc.vector.tensor_scalar_mul(dst, et[:, :],
                                                            kap)
                            else:
                                nc.vector.scalar_tensor_tensor(
                                    dst, et[:, :], kap, dst,
                                    ALU.mult, ALU.add)

            # ---- z init (K branch) ----
            z = [[None] * NT for _ in range(B_LOC)]
            for i in range(B_LOC):
                for t in range(NT):
                    zt = z_pool.tile([128, HW], F32, tag=f"z{i}{t}",
                                     name=f"z{i}{t}", bufs=1)
                    nc.vector.tensor_scalar_mul(
                        zt[:, :], rd(xq[i][t][:, :, 1:W + 1]),
                        chains[i]["sk"][:, t:t + 1])
                    z[i][t] = zt

            # ---- conv + drains + per-sample stats + AllReduce ----
            st_in = [None] * B_LOC
            st_out = [None] * B_LOC
            for i in range(B_LOC):
                for ct in range(NT):
                    for grp in range(2):
                        banks = []
                        for j in range(4):
                            banks.append(ps_conv.tile([128, 8, W], F32,
                                                      tag="cps",
                                                      name=f"cps{j}"))
                        for cit in range(NT):
                            for (dp, dq) in SHIFTS:
                                pq = (dp + 1) * 3 + (dq + 1)
                                lo = pq * C + ct * 128
                                lhs = aggb[i][cit][:, lo:lo + 128]
                                for j in range(4):
                                    chunk = grp * 4 + j
                                    y0 = chunk * 8
                                    ylo = max(y0, -dp)
                                    yhi = min(y0 + 7, H - 1 - dp)
                                    n_r = yhi - ylo + 1
                                    if n_r <= 0:
                                        continue
                                    out_ap = banks[j][:, ylo - y0:
                                                      ylo - y0 + n_r, :]
                                    in_ap = xq[i][cit][:, ylo + dp:
                                                       ylo + dp + n_r,
                                                       1 + dq:1 + dq + W]
                                    first = (cit == 0 and dp == 0 and dq == 0)
                                    last = (cit == NT - 1
                                            and (dp, dq) == SHIFTS[-1])
                                    nc.tensor.matmul(out_ap, lhs, in_ap,
                                                     start=first, stop=last,
                                                     skip_group_check=True)
                        # drain: z = psum * out_att + z
                        for j in range(4):
                            chunk = grp * 4 + j
                            zsl = z[i][ct][:, chunk * 512:(chunk + 1) * 512]
                            nc.vector.scalar_tensor_tensor(
                                zsl, banks[j][:, :, :],
                                chains[i]["oatt"][:, ct:ct + 1], zsl,
                                ALU.mult, ALU.add)
                    # per (i, ct) batchnorm partial stats
                    st = sp.tile([128, 8, 6], F32, tag="bnst")
                    for j in range(8):
                        nc.vector.bn_stats(out=st[:, j, :],
                                           in_=z[i][ct][:, j * 512:
                                                        (j + 1) * 512])
                    mv = sp.tile([128, 2], F32, tag="mv")
                    nc.vector.bn_aggr(out=mv[:, :], in_=st[:, :, :])
                    m2 = sp.tile([128, 1], F32, tag="m2")
                    nc.vector.tensor_mul(m2[:, :], mv[:, 0:1], mv[:, 0:1])
                    ex2 = sp.tile([128, 1], F32, tag="ex2")
                    nc.vector.tensor_add(ex2[:, :], mv[:, 1:2], m2[:, :])
                    cs = 2 * ct
                    nc.vector.tensor_scalar_mul(
                        sa[i][:, cs:cs + 1], mv[:, 0:1], float(HW))
                    nc.vector.tensor_scalar_mul(
                        sa[i][:, cs + 1:cs + 2], ex2[:, :], float(HW))
                # per-sample AllReduce; sample 0's hides under conv of s.1
                st_in[i] = dram.tile([128, 2 * NT], F32, tag=f"st_in{i}",
                                     name=f"st_in{i}", bufs=1)
                st_out[i] = dram.tile([128, 2 * NT], F32, tag=f"st_out{i}",
                                      name=f"st_out{i}", bufs=1)
                nc.gpsimd.dma_start(out=st_in[i][:, :], in_=sa[i][:, :])
                nc.gpsimd.collective_compute(
                    "AllReduce", ALU.add,
                    replica_groups=[list(range(N_CORES))],
                    ins=[st_in[i][:, :].opt()], outs=[st_out[i][:, :].opt()])
                nc.gpsimd.dma_start(out=sg[i][:, :], in_=st_out[i][:, :])

            # ---- finalize BN, relu, write out (permuted channels) ----
            stats_g = singles.tile([128, 2 * NT], F32, tag="stats_g")
            nc.vector.tensor_add(stats_g[:, :], sg[0][:, :], sg[1][:, :])
            out_view = out_d[:, :, :, :].rearrange(
                "b (cl cr) h w -> b cr cl (h w)", cr=4)
            n_total = float(B * HW)
            for t in range(NT):
                mean = sp.tile([128, 1], F32, tag="mean")
                ex2g = sp.tile([128, 1], F32, tag="ex2g")
                nc.vector.tensor_scalar_mul(mean[:, :],
                                            stats_g[:, 2 * t:2 * t + 1],
                                            1.0 / n_total)
                nc.vector.tensor_scalar_mul(ex2g[:, :],
                                            stats_g[:, 2 * t + 1:2 * t + 2],
                                            1.0 / n_total)
                m2g = sp.tile([128, 1], F32, tag="m2g")
                nc.vector.tensor_mul(m2g[:, :], mean[:, :], mean[:, :])
                var = sp.tile([128, 1], F32, tag="var")
                nc.vector.tensor_sub(var[:, :], ex2g[:, :], m2g[:, :])
                rstd = sp.tile([128, 1], F32, tag="rstd")
                nc.scalar.activation(out=rstd[:, :], in_=var[:, :],
                                     func=ACT.Sqrt, bias=eps_t[:, :])
                nc.vector.reciprocal(out=rstd[:, :], in_=rstd[:, :])
                scl = sp.tile([128, 1], F32, tag="scl")
                nc.vector.tensor_mul(scl[:, :], gb_s[:, t, 0:1], rstd[:, :])
                tmpb = sp.tile([128, 1], F32, tag="tmpb")
                nc.vector.tensor_mul(tmpb[:, :], mean[:, :], scl[:, :])
                bia = sp.tile([128, 1], F32, tag="bia")
                nc.vector.tensor_sub(bia[:, :], gb_s[:, t, 1:2], tmpb[:, :])
                for i in range(B_LOC):
                    # split normalize per half so DMA-out starts earlier
                    for ph in range(2):
                        hsl = z[i][t][ph * 64:(ph + 1) * 64, :]
                        nc.scalar.activation(
                            out=hsl, in_=hsl, func=ACT.Relu,
                            bias=bia[ph * 64:(ph + 1) * 64, :],
                            scale=scl[ph * 64:(ph + 1) * 64, :])
                        nc.sync.dma_start(
                            out=out_view[i, 2 * t + ph, :, :],
                            in_=hsl)
    nc.finalize()
    return nc


def _host_prep(inputs):
    """Numpy-side weight re-layouts (all small except ede transpose)."""
    c = np.arange(C)
    pinv = (c % 64) * 4 + c // 64          # output-channel permutation
    ede = np.ascontiguousarray(inputs["ede_weight"], dtype=np.float32)
    ede_p = ede[:, pinv]                    # permute co axis
    # -> [k, ci, pq, co] so an SBUF agg tile is [ci_part, pq*256+co]
    e_host = np.ascontiguousarray(
        ede_p.transpose(0, 2, 3, 4, 1).reshape(KNUM, NT, 128, EFREE))
    if CONV_BF16:
        e_host = e_host.astype(ml_dtypes.bfloat16)
    aow = np.ascontiguousarray(
        inputs["att_out_w"][pinv].T.reshape(NT, 128, C), dtype=np.float32)
    akw = np.ascontiguousarray(
        inputs["att_kernel_w"].T.reshape(NT, 128, KNUM), dtype=np.float32)
    w1t = np.ascontiguousarray(inputs["lga_mlp_w1"].T, dtype=np.float32)
    w2t = np.ascontiguousarray(inputs["lga_mlp_w2"].T, dtype=np.float32)
    gb = np.stack([np.asarray(inputs["bn_gamma"])[pinv].reshape(NT, 128),
                   np.asarray(inputs["bn_beta"])[pinv].reshape(NT, 128)],
                  axis=-1).astype(np.float32)
    aw = np.asarray(inputs["att_conv1d_w"], dtype=np.float32) / HW
    lw = np.asarray(inputs["lga_conv1d_w"], dtype=np.float32) / HW
    lb = float(np.asarray(inputs["lga_conv1d_b"]).reshape(-1)[0])
    sm = np.array([[aw[0], aw[1], aw[2], lw[0], lw[1], lw[2], lb, 0.0]],
                  dtype=np.float32)
    return e_host, aow, akw, w1t, w2t, gb, sm


_CACHE = {}
last_results = None


def _enable_axon_trace():
    """Register the NTFF profile hook that the agent image leaves out."""
    import sys
    import types

    import concourse.bass_utils as bu
    if "antenv.axon_hooks" in sys.modules:
        return
    from trn_agent_boot.trn_boot import _ntff_profile_via_ctypes
    hook = _ntff_profile_via_ctypes("/opt/axon/libaxon_pjrt.so")
    mod = types.ModuleType("antenv.axon_hooks")
    mod.get_axon_ntff_profile_hook = lambda: hook
    mod.set_axon_ntff_profile_hook = lambda h: None
    sys.modules["antenv.axon_hooks"] = mod
    bu.upload_artifacts = lambda tmpdir: f"local:{tmpdir}"


def kernel(_trace=False, _tmpdir=None, **inputs):
    global last_results
    if _trace:
        _enable_axon_trace()
    x = np.asarray(inputs["x"], dtype=np.float32)
    xpad = np.zeros((B, C, H, XW), np.float32)
    xpad[:, :, :, 1:W + 1] = x
    xpad = np.ascontiguousarray(xpad)
    e_host, aow, akw, w1t, w2t, gb, sm = _host_prep(inputs)

    if "nc" not in _CACHE:
        _CACHE["nc"] = build_program()
    nc = _CACHE["nc"]

    shared = {"ew": e_host, "aow": aow, "akw": akw, "w1t": w1t,
              "w2t": w2t, "gb": gb, "sm": sm}
    in_maps = []
    for core in range(N_CORES):
        m = dict(shared)
        m["x"] = xpad[core * B_LOC:(core + 1) * B_LOC]
        in_maps.append(m)

    res = run_bass_kernel_spmd(nc, in_maps, list(range(N_CORES)),
                               trace=_trace, tmpdir=_tmpdir)
    last_results = res
    out = np.concatenate([res.results[i]["out"] for i in range(N_CORES)],
                         axis=0)
    return out


# revision 19
# speedup vs baseline: 17384.6631x; 1.0251x over previous
"""Trainium2 Bass kernel for the DEAM dense-CNN block.

Data-parallel over batch: 16 samples -> 8 cores x 2 samples.
Per sample: attention chain (GAP -> conv1d -> sigmoid/softmax heads),
dynamic per-sample 3x3 conv as 9 shifted matmuls, LGA gating branch,
fused add + batch BN (cross-core AllReduce of per-channel sums) + ReLU.

Channel shuffle is folded into host-side weight permutations plus a
permuted output DMA, so no on-chip data movement is spent on it.

v2: conv runs in bf16 (fp32 streams at ~2 cycles/row on the PE; bf16
about 2x faster). Dynamic-weight build + GAP/max reductions run on
GpSimd so the Vector engine keeps up with PSUM drains. Per-sample BN
AllReduce: sample 0's reduction hides under sample 1's conv.
"""

import numpy as np
import ml_dtypes

import concourse.bass as bass
import concourse.mybir as mybir
import concourse.tile as tile
from concourse import bacc
from concourse.bass_utils import run_bass_kernel_spmd
from concourse.masks import make_identity

F32 = mybir.dt.float32
F32R = mybir.dt.float32r
BF16 = mybir.dt.bfloat16
AX = mybir.AxisListType
ALU = mybir.AluOpType
ACT = mybir.ActivationFunctionType

B, C, H, W = 16, 256, 64, 64
HW = H * W
KNUM, KS = 4, 3
N_CORES = 8
B_LOC = B // N_CORES          # 2 samples per core
NT = C // 128                 # 2 channel tiles
BN_EPS = 1e-5
XW = W + 2                    # padded row width 66
PQ = KS * KS                  # 9
EFREE = PQ * C                # 2304 free elems of an agg/E tile
EHALF = EFREE // 2            # 1152

CONV_BF16 = True              # bf16 conv (2x PE throughput) vs fp32r

# shift order: (0,0) first so the start=True matmul covers the full bank
SHIFTS = [(0, 0), (0, -1), (0, 1), (-1, -1), (-1, 0), (-1, 1),
          (1, -1), (1, 0), (1, 1)]


def build_program():
    cdt = BF16 if CONV_BF16 else F32R
    xdt = BF16 if CONV_BF16 else F32R
    edt = BF16 if CONV_BF16 else F32

    nc = bacc.Bacc("TRN2", target_bir_lowering=False, debug=False,
                   num_devices=N_CORES)

    x_d = nc.dram_tensor("x", [B_LOC, C, H, XW], xdt, kind="ExternalInput")
    e_d = nc.dram_tensor("ew", [KNUM, NT, 128, EFREE], edt,
                         kind="ExternalInput")
    aow_d = nc.dram_tensor("aow", [NT, 128, C], F32, kind="ExternalInput")
    akw_d = nc.dram_tensor("akw", [NT, 128, KNUM], F32, kind="ExternalInput")
    w1t_d = nc.dram_tensor("w1t", [128, 16], F32, kind="ExternalInput")
    w2t_d = nc.dram_tensor("w2t", [16, 128], F32, kind="ExternalInput")
    gb_d = nc.dram_tensor("gb", [NT, 128, 2], F32, kind="ExternalInput")
    sm_d = nc.dram_tensor("sm", [1, 8], F32, kind="ExternalInput")
    out_d = nc.dram_tensor("out", [B_LOC, C, H, W], F32,
                           kind="ExternalOutput")

    with tile.TileContext(nc) as tc:
        with (
            tc.tile_pool(name="singles", bufs=1) as singles,
            tc.tile_pool(name="xq", bufs=1) as xq_pool,
            tc.tile_pool(name="z", bufs=1) as z_pool,
            tc.tile_pool(name="ep", bufs=16) as e_pool,
            tc.tile_pool(name="aggb", bufs=1) as aggb_pool,
            tc.tile_pool(name="sp", bufs=2) as sp,
            tc.tile_pool(name="psc", bufs=6, space="PSUM") as ps_conv,
            tc.tile_pool(name="pss", bufs=2, space="PSUM") as pss,
            tc.tile_pool(name="dram", bufs=4, space="DRAM") as dram,
        ):
            # ---- constants ----
            ident = singles.tile([128, 128], F32, tag="ident")
            make_identity(nc, ident[:, :])
            smalls = singles.tile([1, 8], F32, tag="smalls")
            nc.sync.dma_start(out=smalls[:, :], in_=sm_d[:, :])
            aow_s = []
            akw_s = []
            for t in range(NT):
                a = singles.tile([128, C], F32, tag=f"aow{t}")
                nc.sync.dma_start(out=a[:, :], in_=aow_d[t])
                aow_s.append(a)
                k = singles.tile([128, KNUM], F32, tag=f"akw{t}")
                nc.sync.dma_start(out=k[:, :], in_=akw_d[t])
                akw_s.append(k)
            w1t_s = singles.tile([128, 16], F32, tag="w1t")
            nc.sync.dma_start(out=w1t_s[:, :], in_=w1t_d[:, :])
            w2t_s = singles.tile([16, 128], F32, tag="w2t")
            nc.sync.dma_start(out=w2t_s[:, :], in_=w2t_d[:, :])
            gb_s = singles.tile([128, NT, 2], F32, tag="gb")
            for t in range(NT):
                nc.sync.dma_start(out=gb_s[:, t, :], in_=gb_d[t])
            eps_t = singles.tile([128, 1], F32, tag="eps_t")
            nc.vector.memset(eps_t[:, :], BN_EPS)
            ones1 = singles.tile([1, 128], F32, tag="ones1")
            nc.vector.memset(ones1[:, :], 1.0)
            # per-sample stats (sum, sumsq) x (tile0, tile1) + AR results
            sa = [singles.tile([128, 2 * NT], F32, tag=f"sa{i}",
                               name=f"sa{i}") for i in range(B_LOC)]
            sg = [singles.tile([128, 2 * NT], F32, tag=f"sg{i}",
                               name=f"sg{i}") for i in range(B_LOC)]

            # ---- load x (already bf16 from host when CONV_BF16) ----
            dma_x = nc.sync
            xq = [[None] * NT for _ in range(B_LOC)]
            for i in range(B_LOC):
                for t in range(NT):
                    xt = xq_pool.tile([128, H, XW], cdt, tag=f"xq{i}{t}",
                                      name=f"xq{i}{t}", bufs=1)
                    dma_x.dma_start(out=xt[:, :, :],
                                    in_=x_d[i, t * 128:(t + 1) * 128])
                    xq[i][t] = xt

            def rd(ap):
                return ap if CONV_BF16 else ap.bitcast(F32)

            # ---- per-sample attention / gating chain ----
            chains = []
            for i in range(B_LOC):
                ch = {}
                gsum = sp.tile([128, NT], F32, tag="gsum")
                tmp64 = sp.tile([128, H], F32, tag="tmp64")
                for t in range(NT):
                    nc.vector.tensor_reduce(tmp64[:, :],
                                            rd(xq[i][t][:, :, 1:W + 1]),
                                            axis=AX.X, op=ALU.add)
                    nc.vector.tensor_reduce(gsum[:, t:t + 1], tmp64[:, :],
                                            axis=AX.X, op=ALU.add)
                # max over in2 (tile 1) for the LGA mlp
                vv = sp.tile([128, 2], F32, tag="vv")
                tmp64b = sp.tile([128, H], F32, tag="tmp64b")
                nc.vector.tensor_reduce(tmp64b[:, :],
                                        rd(xq[i][1][:, :, 1:W + 1]),
                                        axis=AX.X, op=ALU.max)
                nc.vector.tensor_reduce(vv[:, 0:1], tmp64b[:, :],
                                        axis=AX.X, op=ALU.max)
                nc.vector.tensor_scalar_mul(vv[:, 1:2], gsum[:, 1:2],
                                            1.0 / HW)

                # gap -> free layout (two (128,1) -> (1,128) transposes)
                gf = []
                for t in range(NT):
                    g_ps = pss.tile([1, 128], F32, tag="pst")
                    nc.tensor.transpose(g_ps[:, :], gsum[:, t:t + 1],
                                        ident[:, :])
                    gf.append(g_ps)
                g2 = sp.tile([1, C + 2], F32, tag="g2")
                nc.vector.memset(g2[:, :], 0.0)
                for t in range(NT):
                    nc.vector.tensor_copy(
                        out=g2[0:1, 1 + t * 128:1 + (t + 1) * 128],
                        in_=gf[t][0:1, :])
                gs = sp.tile([1, 130], F32, tag="gs")
                nc.vector.memset(gs[:, :], 0.0)
                nc.vector.tensor_copy(out=gs[0:1, 1:129], in_=gf[0][0:1, :])

                # t = conv1d(gap_mean, att_w) : weights pre-scaled by 1/HW
                ta = sp.tile([1, C], F32, tag="ta")
                tb = sp.tile([1, C], F32, tag="tb")
                t_t = sp.tile([1, C], F32, tag="t_t")
                nc.vector.tensor_scalar_mul(ta[:, :], g2[0:1, 0:C],
                                            smalls[0:1, 0:1])
                nc.vector.scalar_tensor_tensor(tb[:, :], g2[0:1, 1:C + 1],
                                               smalls[0:1, 1:2], ta[:, :],
                                               ALU.mult, ALU.add)
                nc.vector.scalar_tensor_tensor(t_t[:, :], g2[0:1, 2:C + 2],
                                               smalls[0:1, 2:3], tb[:, :],
                                               ALU.mult, ALU.add)

                # s = conv1d(gap1_mean, lga_w) + b (bias added in free layout)
                sa_ = sp.tile([1, 128], F32, tag="sa_")
                sb_ = sp.tile([1, 128], F32, tag="sb_")
                s_t = sp.tile([1, 128], F32, tag="s_t")
                nc.vector.tensor_scalar_mul(sa_[:, :], gs[0:1, 0:128],
                                            smalls[0:1, 3:4])
                nc.vector.scalar_tensor_tensor(sb_[:, :], gs[0:1, 1:129],
                                               smalls[0:1, 4:5], sa_[:, :],
                                               ALU.mult, ALU.add)
                nc.vector.scalar_tensor_tensor(s_t[:, :], gs[0:1, 2:130],
                                               smalls[0:1, 5:6], sb_[:, :],
                                               ALU.mult, ALU.add)
                nc.vector.tensor_scalar_add(s_t[:, :], s_t[:, :],
                                            smalls[0:1, 6:7])

                # transposes back to partition layout
                tps = sp.tile([128, NT], F32, tag="tps")
                ia = sp.tile([128, NT], F32, tag="ia")
                for t in range(NT):
                    tp_ps = pss.tile([128, 1], F32, tag="pst")
                    nc.tensor.transpose(tp_ps[:, :],
                                        t_t[0:1, t * 128:(t + 1) * 128],
                                        ident[0:1, 0:1])
                    nc.vector.tensor_copy(out=tps[:, t:t + 1],
                                          in_=tp_ps[:, :])
                    nc.scalar.activation(out=ia[:, t:t + 1], in_=tp_ps[:, :],
                                         func=ACT.Sigmoid)
                sk = sp.tile([128, 2], F32, tag="sk")
                sp_ps = pss.tile([128, 1], F32, tag="pst")
                nc.tensor.transpose(sp_ps[:, :], s_t[0:1, :],
                                    ident[0:1, 0:1])
                nc.scalar.activation(out=sk[:, 0:1], in_=sp_ps[:, :],
                                     func=ACT.Sigmoid)

                # out_att (permuted) per co tile
                oatt = sp.tile([128, NT], F32, tag="oatt")
                for ct in range(NT):
                    o_ps = pss.tile([128, 1], F32, tag="pst")
                    for t in range(NT):
                        nc.tensor.matmul(
                            o_ps[:, :],
                            aow_s[t][:, ct * 128:(ct + 1) * 128],
                            tps[:, t:t + 1],
                            start=(t == 0), stop=(t == NT - 1))
                    nc.scalar.activation(out=oatt[:, ct:ct + 1],
                                         in_=o_ps[:, :], func=ACT.Sigmoid)

                # kernel attention logits -> softmax -> broadcast
                kl_ps = pss.tile([KNUM, 1], F32, tag="pst")
                for t in range(NT):
                    nc.tensor.matmul(kl_ps[:, :], akw_s[t][:, :],
                                     tps[:, t:t + 1],
                                     start=(t == 0), stop=(t == NT - 1))
                kls = sp.tile([KNUM, 1], F32, tag="kls")
                nc.vector.tensor_copy(out=kls[:, :], in_=kl_ps[:, :])
                kt_ps = pss.tile([1, KNUM], F32, tag="pst")
                nc.tensor.transpose(kt_ps[:, :], kls[:, :],
                                    ident[0:KNUM, 0:KNUM])
                mx = sp.tile([1, 1], F32, tag="mx")
                nc.vector.reduce_max(mx[:, :], kt_ps[0:1, :], axis=AX.X)
                ex = sp.tile([1, KNUM], F32, tag="ex")
                nc.vector.tensor_scalar(out=ex[:, :], in0=kt_ps[0:1, :],
                                        scalar1=mx[:, :], scalar2=None,
                                        op0=ALU.subtract)
                exs = sp.tile([1, KNUM], F32, tag="exs")
                nc.scalar.activation(out=exs[:, :], in_=ex[:, :],
                                     func=ACT.Exp)
                sm1 = sp.tile([1, 1], F32, tag="sm1")
                nc.vector.reduce_sum(sm1[:, :], exs[:, :], axis=AX.X)
                nc.vector.reciprocal(out=sm1[:, :], in_=sm1[:, :])
                katt = sp.tile([1, KNUM], F32, tag="katt")
                nc.vector.tensor_scalar_mul(katt[:, :], exs[:, :],
                                            sm1[:, :])
                kattb = sp.tile([128, KNUM], F32, tag="kattb")
                kb_ps = pss.tile([128, KNUM], F32, tag="pst")
                nc.tensor.matmul(kb_ps[:, :], ones1[:, :], katt[0:1, :],
                                 start=True, stop=True)
                nc.vector.tensor_copy(out=kattb[:, :], in_=kb_ps[:, :])

                # LGA mlp: sigmoid(mlp(max) + mlp(mean))
                h_ps = pss.tile([16, 2], F32, tag="pst")
                nc.tensor.matmul(h_ps[:, :], w1t_s[:, :], vv[:, :],
                                 start=True, stop=True)
                h_s = sp.tile([16, 2], F32, tag="h_s")
                nc.scalar.activation(out=h_s[:, :], in_=h_ps[:, :],
                                     func=ACT.Relu)
                m_ps = pss.tile([128, 2], F32, tag="pst")
                nc.tensor.matmul(m_ps[:, :], w2t_s[:, :], h_s[:, :],
                                 start=True, stop=True)
                mcp = sp.tile([128, 2], F32, tag="mcp")
                nc.vector.tensor_copy(out=mcp[:, :], in_=m_ps[:, :])
                chadd = sp.tile([128, 1], F32, tag="chadd")
                nc.vector.tensor_add(chadd[:, :], mcp[:, 0:1], mcp[:, 1:2])
                nc.scalar.activation(out=sk[:, 1:2], in_=chadd[:, :],
                                     func=ACT.Sigmoid)

                # fold in_att into the softmax scalars: kia[ci,k] = katt_k*ia_t[ci]
                kia = sp.tile([128, NT, KNUM], F32, tag="kia")
                for t in range(NT):
                    nc.vector.tensor_scalar_mul(kia[:, t, :], kattb[:, :],
                                                ia[:, t:t + 1])
                ch["kia"] = kia
                ch["kattb"] = kattb
                ch["ia"] = ia
                ch["oatt"] = oatt
                ch["sk"] = sk
                chains.append(ch)

            # ---- dynamic conv weights: aggb = bf16(sum_k kia_k E_k)
            # (in_att folded into kia); E streamed once, MACs on DVE
            aggb = [[None] * NT for _ in range(B_LOC)]
            for i in range(B_LOC):
                for t in range(NT):
                    aggb[i][t] = aggb_pool.tile(
                        [128, EFREE], cdt, tag=f"aggb{i}{t}",
                        name=f"aggb{i}{t}", bufs=1)
            e_tiles = {}
            for t in range(NT):
                for k in range(KNUM):
                    for hh in range(2):
                        et = e_pool.tile([128, EHALF], edt, tag="e")
                        nc.sync.dma_start(
                            out=et[:, :],
                            in_=e_d[k, t, :, hh * EHALF:(hh + 1) * EHALF])
                        e_tiles[(t, k, hh)] = et
            for i in range(B_LOC):
                for t in range(NT):
                    for k in range(KNUM):
                        for hh in range(2):
                            et = e_tiles[(t, k, hh)]
                            dst = aggb[i][t][:, hh * EHALF:(hh + 1) * EHALF]
                            kap = chains[i]["kia"][:, t, k:k + 1]
                            if k == 0:
                                nc.vector.tensor_scalar_mul(dst, et[:, :],
                                                            kap)
                            else:
                                nc.vector.scalar_tensor_tensor(
                                    dst, et[:, :], kap, dst,
                                    ALU.mult, ALU.add)

            # ---- z init (K branch) ----
            z = [[None] * NT for _ in range(B_LOC)]
            for i in range(B_LOC):
                for t in range(NT):
                    zt = z_pool.tile([128, HW], F32, tag=f"z{i}{t}",
                                     name=f"z{i}{t}", bufs=1)
                    nc.vector.tensor_scalar_mul(
                        zt[:, :], rd(xq[i][t][:, :, 1:W + 1]),
                        chains[i]["sk"][:, t:t + 1])
                    z[i][t] = zt

            # ---- conv + drains + per-sample stats + AllReduce ----
            st_in = [None] * B_LOC
            st_out = [None] * B_LOC
            for i in range(B_LOC):
                for ct in range(NT):
                    for grp in range(2):
                        banks = []
                        for j in range(4):
                            banks.append(ps_conv.tile([128, 8, W], F32,
                                                      tag="cps",
                                                      name=f"cps{j}"))
                        for cit in range(NT):
                            for (dp, dq) in SHIFTS:
                                pq = (dp + 1) * 3 + (dq + 1)
                                lo = pq * C + ct * 128
                                lhs = aggb[i][cit][:, lo:lo + 128]
                                for j in range(4):
                                    chunk = grp * 4 + j
                                    y0 = chunk * 8
                                    ylo = max(y0, -dp)
                                    yhi = min(y0 + 7, H - 1 - dp)
                                    n_r = yhi - ylo + 1
                                    if n_r <= 0:
                                        continue
                                    out_ap = banks[j][:, ylo - y0:
                                                      ylo - y0 + n_r, :]
                                    in_ap = xq[i][cit][:, ylo + dp:
                                                       ylo + dp + n_r,
                                                       1 + dq:1 + dq + W]
                                    first = (cit == 0 and dp == 0 and dq == 0)
                                    last = (cit == NT - 1
                                            and (dp, dq) == SHIFTS[-1])
                                    nc.tensor.matmul(out_ap, lhs, in_ap,
                                                     start=first, stop=last,
                                                     skip_group_check=True)
                        # drain: z = psum * out_att + z
                        for j in range(4):
                            chunk = grp * 4 + j
                            zsl = z[i][ct][:, chunk * 512:(chunk + 1) * 512]
                            nc.vector.scalar_tensor_tensor(
                                zsl, banks[j][:, :, :],
                                chains[i]["oatt"][:, ct:ct + 1], zsl,
                                ALU.mult, ALU.add)
                    # per (i, ct) batchnorm partial stats
                    st = sp.tile([128, 8, 6], F32, tag="bnst")
                    for j in range(8):
                        nc.vector.bn_stats(out=st[:, j, :],
                                           in_=z[i][ct][:, j * 512:
                                                        (j + 1) * 512])
                    mv = sp.tile([128, 2], F32, tag="mv")
                    nc.vector.bn_aggr(out=mv[:, :], in_=st[:, :, :])
                    m2 = sp.tile([128, 1], F32, tag="m2")
                    nc.vector.tensor_mul(m2[:, :], mv[:, 0:1], mv[:, 0:1])
                    ex2 = sp.tile([128, 1], F32, tag="ex2")
                    nc.vector.tensor_add(ex2[:, :], mv[:, 1:2], m2[:, :])
                    cs = 2 * ct
                    nc.vector.tensor_scalar_mul(
                        sa[i][:, cs:cs + 1], mv[:, 0:1], float(HW))
                    nc.vector.tensor_scalar_mul(
                        sa[i][:, cs + 1:cs + 2], ex2[:, :], float(HW))
                # per-sample AllReduce; sample 0's hides under conv of s.1
                st_in[i] = dram.tile([128, 2 * NT], F32, tag=f"st_in{i}",
                                     name=f"st_in{i}", bufs=1)
                st_out[i] = dram.tile([128, 2 * NT], F32, tag=f"st_out{i}",
                                      name=f"st_out{i}", bufs=1)
                nc.gpsimd.dma_start(out=st_in[i][:, :], in_=sa[i][:, :])
                nc.gpsimd.collective_compute(
                    "AllReduce", ALU.add,
                    replica_groups=[list(range(N_CORES))],
                    ins=[st_in[i][:, :].opt()], outs=[st_out[i][:, :].opt()])
                nc.gpsimd.dma_start(out=sg[i][:, :], in_=st_out[i][:, :])

            # ---- finalize BN, relu, write out (permuted channels) ----
            stats_g = singles.tile([128, 2 * NT], F32, tag="stats_g")
            nc.vector.tensor_add(stats_g[:, :], sg[0][:, :], sg[1][:, :])
            out_view = out_d[:, :, :, :].rearrange(
                "b (cl cr) h w -> b cr cl (h w)", cr=4)
            n_total = float(B * HW)
            for t in range(NT):
                mean = sp.tile([128, 1], F32, tag="mean")
                ex2g = sp.tile([128, 1], F32, tag="ex2g")
                nc.vector.tensor_scalar_mul(mean[:, :],
                                            stats_g[:, 2 * t:2 * t + 1],
                                            1.0 / n_total)
                nc.vector.tensor_scalar_mul(ex2g[:, :],
                                            stats_g[:, 2 * t + 1:2 * t + 2],
                                            1.0 / n_total)
                m2g = sp.tile([128, 1], F32, tag="m2g")
                nc.vector.tensor_mul(m2g[:, :], mean[:, :], mean[:, :])
                var = sp.tile([128, 1], F32, tag="var")
                nc.vector.tensor_sub(var[:, :], ex2g[:, :], m2g[:, :])
                rstd = sp.tile([128, 1], F32, tag="rstd")
                nc.scalar.activation(out=rstd[:, :], in_=var[:, :],
                                     func=ACT.Sqrt, bias=eps_t[:, :])
                nc.vector.reciprocal(out=rstd[:, :], in_=rstd[:, :])
                scl = sp.tile([128, 1], F32, tag="scl")
                nc.vector.tensor_mul(scl[:, :], gb_s[:, t, 0:1], rstd[:, :])
                tmpb = sp.tile([128, 1], F32, tag="tmpb")
                nc.vector.tensor_mul(tmpb[:, :], mean[:, :], scl[:, :])
                bia = sp.tile([128, 1], F32, tag="bia")
                nc.vector.tensor_sub(bia[:, :], gb_s[:, t, 1:2], tmpb[:, :])
                for i in range(B_LOC):
                    # normalize split across ACT and DVE, per half-tile
                    for ph in range(2):
                        hsl = z[i][t][ph * 64:(ph + 1) * 64, :]
                        unit = t * 4 + i * 2 + ph
                        if unit in (2, 5, 7):
                            nc.vector.tensor_scalar(
                                out=hsl, in0=hsl,
                                scalar1=scl[ph * 64:(ph + 1) * 64, :],
                                scalar2=bia[ph * 64:(ph + 1) * 64, :],
                                op0=ALU.mult, op1=ALU.add)
                            nc.vector.tensor_relu(out=hsl, in_=hsl)
                        else:
                            nc.scalar.activation(
                                out=hsl, in_=hsl, func=ACT.Relu,
                                bias=bia[ph * 64:(ph + 1) * 64, :],
                                scale=scl[ph * 64:(ph + 1) * 64, :])
                        nc.sync.dma_start(
                            out=out_view[i, 2 * t + ph, :, :],
                            in_=hsl)
    nc.finalize()
    return nc


def _host_prep(inputs):
    """Numpy-side weight re-layouts (all small except ede transpose)."""
    c = np.arange(C)
    pinv = (c % 64) * 4 + c // 64          # output-channel permutation
    ede = np.ascontiguousarray(inputs["ede_weight"], dtype=np.float32)
    ede_p = ede[:, pinv]                    # permute co axis
    # -> [k, ci, pq, co] so an SBUF agg tile is [ci_part, pq*256+co]
    e_host = np.ascontiguousarray(
        ede_p.transpose(0, 2, 3, 4, 1).reshape(KNUM, NT, 128, EFREE))
    if CONV_BF16:
        e_host = e_host.astype(ml_dtypes.bfloat16)
    aow = np.ascontiguousarray(
        inputs["att_out_w"][pinv].T.reshape(NT, 128, C), dtype=np.float32)
    akw = np.ascontiguousarray(
        inputs["att_kernel_w"].T.reshape(NT, 128, KNUM), dtype=np.float32)
    w1t = np.ascontiguousarray(inputs["lga_mlp_w1"].T, dtype=np.float32)
    w2t = np.ascontiguousarray(inputs["lga_mlp_w2"].T, dtype=np.float32)
    gb = np.stack([np.asarray(inputs["bn_gamma"])[pinv].reshape(NT, 128),
                   np.asarray(inputs["bn_beta"])[pinv].reshape(NT, 128)],
                  axis=-1).astype(np.float32)
    aw = np.asarray(inputs["att_conv1d_w"], dtype=np.float32) / HW
    lw = np.asarray(inputs["lga_conv1d_w"], dtype=np.float32) / HW
    lb = float(np.asarray(inputs["lga_conv1d_b"]).reshape(-1)[0])
    sm = np.array([[aw[0], aw[1], aw[2], lw[0], lw[1], lw[2], lb, 0.0]],
                  dtype=np.float32)
    return e_host, aow, akw, w1t, w2t, gb, sm


_CACHE = {}
last_results = None


def _enable_axon_trace():
    """Register the NTFF profile hook that the agent image leaves out."""
    import sys
    import types

    import concourse.bass_utils as bu
    if "antenv.axon_hooks" in sys.modules:
        return
    from trn_agent_boot.trn_boot import _ntff_profile_via_ctypes
    hook = _ntff_profile_via_ctypes("/opt/axon/libaxon_pjrt.so")
    mod = types.ModuleType("antenv.axon_hooks")
    mod.get_axon_ntff_profile_hook = lambda: hook
    mod.set_axon_ntff_profile_hook = lambda h: None
    sys.modules["antenv.axon_hooks"] = mod
    bu.upload_artifacts = lambda tmpdir: f"local:{tmpdir}"


def kernel(_trace=False, _tmpdir=None, **inputs):
    global last_results
    if _trace:
        _enable_axon_trace()
    x = np.asarray(inputs["x"], dtype=np.float32)
    xpad = np.zeros((B, C, H, XW), np.float32)
    xpad[:, :, :, 1:W + 1] = x
    if CONV_BF16:
        xpad = xpad.astype(ml_dtypes.bfloat16)
    xpad = np.ascontiguousarray(xpad)
    e_host, aow, akw, w1t, w2t, gb, sm = _host_prep(inputs)

    if "nc" not in _CACHE:
        _CACHE["nc"] = build_program()
    nc = _CACHE["nc"]

    shared = {"ew": e_host, "aow": aow, "akw": akw, "w1t": w1t,
              "w2t": w2t, "gb": gb, "sm": sm}
    in_maps = []
    for core in range(N_CORES):
        m = dict(shared)
        m["x"] = xpad[core * B_LOC:(core + 1) * B_LOC]
        in_maps.append(m)

    res = run_bass_kernel_spmd(nc, in_maps, list(range(N_CORES)),
                               trace=_trace, tmpdir=_tmpdir)
    last_results = res
    out = np.concatenate([res.results[i]["out"] for i in range(N_CORES)],
                         axis=0)
    return out
